# revision 14
# baseline (speedup 1.0000x reference)
"""Trainium2 Bass kernel for nn_MultiHeadAttention_48395691492077.

Reference (B=4, S=2048, D=1024, single head, anti-causal triu mask):
    qkv = x @ wqkv; q,k,v = split(qkv)
    scores = triu(q @ k^T / sqrt(B));  masked softmax over keys t >= s
    x2  = softmax(scores) @ v @ w_lin + b_lin + x
    out = relu(x2 @ w_ff1 + b_ff1) @ w_ff2 + b_ff2 + x2

Sharding: 8 cores = 4 batches x 2 query-halves. Each core computes
attention + MLP for its own 1024 queries against the full 2048-key
sequence of its batch. The program is identical on all cores (SPMD);
per-core differences (which queries, which mask pattern) are carried in
the input data plus one branch on the query-parity register.

Device algebra (transposed; no on-chip transposes, no K/V projections):
    uT = wzq^T.T @ qxT            with wzq = (Wq @ Wk^T)/2  (host-fused)
    scoresT[t,s] = sum_d xT[d,t] * uT[d,s]     (keys are raw x!)
    expT = exp(scoresT); diagonal 128-strips *= lower-tri mask
    den[s]/128 broadcast = (ones/128).T @ expT (PE, inlined into pass 1)
    et8 = fp8(expT * 128/den)   (row-stochastic weights x128, fp8 range-safe)
    den2[s] = ones8.T @ et8     (fp8 DoubleRow; renormalizes the fp8
          quantization so attention stays exactly row-stochastic)
    H^T[d,s] = xn8[t,d].T @ et8 (A@X in fp8 DoubleRow; V proj deferred)
    attnT = H^T / den2
    x2T = wvl.T @ attnT + (xT + b_lin)  with wvl = Wv @ w_lin (host-fused:
          A@(X@Wv)@w_lin == (A@X)@(Wv@w_lin) by associativity)
    hT = relu(w_ff1.T @ x2T + b_ff1)
    outT = w_ff2.T @ hT + x2T               (+ b_ff2 added on host)
Blocks crossing the anti-causal diagonal use exact matmul widths
(128/256/384) instead of full 512; PSUM accumulation runs widest-first
so every column is initialized by the start=True matmul. A@X pairs
adjacent t-chunks for DoubleRow; diagonal blocks' tail columns are
zero-padded once so paired reads stay correct.
Matmul inputs are bf16 (fp32 PSUM accumulation) except A@X/FFN (fp8
DoubleRow); residuals are fp32. ff2 adds in place into x2f and streams
per-oc output DMAs on alternating queues to keep the tail short.
"""

import numpy as np
import ml_dtypes

B, S, D = 4, 2048, 1024
NCORES = 8
BF16 = ml_dtypes.bfloat16

NT = S // 128            # 16 t-chunks
ND = D // 128            # 8 chunks of 128 along any D-sized dim

# global query-column starts of (sb0, sb1) per parity
SB_GLOBAL = {0: (0, 1536), 1: (512, 1024)}
# t-chunks each (parity, s-block) actually needs (branch-specialized)
SB_SLOTS = {
    p: {sb: list(range(SB_GLOBAL[p][sb] // 128, NT)) for sb in (0, 1)}
    for p in (0, 1)
}
# t-chunk pairs for fp8 DoubleRow (tc starts are even, runs end at NT-1)
SB_PAIRS = {
    p: {sb: list(range(SB_GLOBAL[p][sb] // 256, NT // 2)) for sb in (0, 1)}
    for p in (0, 1)
}


def _width(parity, sb, tc):
    """Valid column count of block (sb, tc): cols [0, w) of the 512."""
    return min(512, 128 * tc - SB_GLOBAL[parity][sb] + 128)


def _is_diag(parity, sb, tc):
    """Block whose last 128 columns lie on the anti-causal diagonal."""
    return tc - SB_GLOBAL[parity][sb] // 128 < 4


_COMPILED = None
_LAST_IN_MAPS = None


def _build_program():
    from contextlib import ExitStack
    import concourse.bacc as bacc
    import concourse.mybir as mybir
    import concourse.tile as tile

    f32 = mybir.dt.float32
    b16 = mybir.dt.bfloat16
    AF = mybir.ActivationFunctionType

    nc = bacc.Bacc("TRN2", target_bir_lowering=False, debug=False,
                   num_devices=NCORES)

    # all big inputs arrive pre-arranged on the host into the on-chip
    # [128, chunk, free] layout so every DMA is contiguous per partition
    f8 = mybir.dt.float8e4
    xT_d = nc.dram_tensor("xT", [128, ND * S], b16, kind="ExternalInput")
    xn_d = nc.dram_tensor("xn", [128, NT * D], f8, kind="ExternalInput")
    qxT_d = nc.dram_tensor("qxT", [128, ND * 1024], b16, kind="ExternalInput")
    xq_d = nc.dram_tensor("xq", [D, 1024], f32, kind="ExternalInput")
    wzq_d = nc.dram_tensor("wzq", [128, ND * D], b16, kind="ExternalInput")
    wvl_d = nc.dram_tensor("wvl", [128, ND * D], b16, kind="ExternalInput")
    wff1_d = nc.dram_tensor("wff1", [128, ND * D], f8, kind="ExternalInput")
    wff2_d = nc.dram_tensor("wff2", [128, ND * D], f8, kind="ExternalInput")
    # ident | Lneg: identity and the strictly-lower -30000 mask-bias, both
    # bf16; the diag mask is applied as one extra PE matmul into the scores
    # PSUM (ident.T @ Lneg adds -30000 below the diagonal) instead of a
    # post-exp vector multiply
    tri_d = nc.dram_tensor("tri", [128, 256], b16, kind="ExternalInput")
    par_d = nc.dram_tensor("par", [1, 1], mybir.dt.uint32, kind="ExternalInput")
    bf1_d = nc.dram_tensor("bf1", [ND, 128], f32, kind="ExternalInput")
    outT_d = nc.dram_tensor("outT", [D, 1024], f32, kind="ExternalOutput")

    with tile.TileContext(nc) as tc:
        es = ExitStack()
        with es:
            pp = es.enter_context(tc.tile_pool(name="persist", bufs=1))
            sp = es.enter_context(tc.tile_pool(name="stream", bufs=2))
            ps = es.enter_context(
                tc.tile_pool(name="ps", bufs=8, space="PSUM"))
            esB = es.enter_context(ExitStack())
            pb = esB.enter_context(tc.tile_pool(name="pB", bufs=1))
            pr = es.enter_context(tc.tile_pool(name="pAC", bufs=1,
                                               side="right"))

            def psum():
                t = ps.tile([128, 512], f32, tag="mm", bufs=6, name="mmps")
                return t

            def psum_den():
                # den/den2 banks stay live across many mm-tag rotations
                return ps.tile([128, 512], f32, tag="den", bufs=2, name="denps")

            # ---- constants ----
            # load the parity register up front so every engine sequencer
            # resolves it during the startup DMA wait, not at the branch
            par_regs = nc.alloc_registers("par_regs")
            nc.regs_load(par_regs, par_d.ap()[0:1, 0:1])
            par = nc.snap(par_regs, donate=True, min_val=0, max_val=1)

            # den accumulates (ones/128).T @ et so rbs = recip gives 128/den,
            # folding the fp8 weight scale (x128) into the reciprocal for free
            ones_sq = pp.tile([128, 128], b16, tag="ones_sq", bufs=1)
            nc.vector.memset(ones_sq[:], 1.0 / 128)
            ones8 = pp.tile([128, 2, 128], f8, tag="ones8", bufs=1)
            nc.vector.memset(ones8[:], 1.0)
            tri_t = pp.tile([128, 256], b16, tag="tri", bufs=1)
            ident_t = tri_t[:, 0:128]
            lneg_t = tri_t[:, 128:256]
            # warm the PE HAM clock-gate while the first input DMAs land
            wups = psum()
            for i in range(16):
                nc.tensor.matmul(wups[:, 0:128], ones_sq[:], ones_sq[:],
                                 start=(i == 0), stop=(i == 15))

            # ---- input loads (arrival-ordered for phase-A pipelining).
            # Descriptor generation serializes per issuing queue (~0.7us per
            # dma_start), so the early loads fan out across engine queues.
            def chunks(dram, c0, c1, width):
                return dram.ap()[:, c0 * width:c1 * width].rearrange(
                    "p (c n) -> p c n", n=width)

            wzq_a = pr.tile([128, ND, D], b16, tag="wzq", bufs=1)
            qx_a = pr.tile([128, ND, 1024], b16, tag="qx", bufs=1)
            # single-a granules so the a-outer phase-A loop starts as soon
            # as wzq[a0] + the sb0 half of qx[a0] land
            for a in range(ND):
                nc.sync.dma_start(wzq_a[:, a:a + 1], chunks(wzq_d, a, a + 1, D))
                nc.sync.dma_start(
                    qx_a[:, a:a + 1, 0:512],
                    qxT_d.ap()[:, a * 1024:a * 1024 + 512]
                    .rearrange("p (c n) -> p c n", n=512))
            nc.sync.dma_start(
                qx_a[:, :, 512:1024],
                qxT_d.ap().rearrange("p (c n) -> p c n", n=1024)[:, :, 512:1024])
            # xT feeds the scores pass; host interleaves it so each 512-col
            # chunk is flat-contiguous (full-bandwidth descriptors). Chunk
            # order serves both parities' first pass-1 blocks, then the
            # descending tail.
            xt_a = pb.tile([128, 4, ND, 512], b16, tag="xt", bufs=1)
            for cc in (1, 0, 3, 2):
                nc.sync.dma_start(
                    xt_a[:, cc],
                    xT_d.ap()[:, cc * 4096:(cc + 1) * 4096]
                    .rearrange("p (a n) -> p a n", n=512))
            # x natural layout [t, d] in fp8 feeds the A@X DoubleRow pass
            xn_a = pb.tile([128, NT, D], f8, tag="xn", bufs=1)
            nc.sync.dma_start(xn_a[:], chunks(xn_d, 0, NT, D))
            nc.sync.dma_start(tri_t[:], tri_d.ap())
            # b_ff1 laid out [128, ND]: bias column fc serves f-chunk fc
            bf1_t = pp.tile([128, ND], f32, tag="bf1", bufs=1)
            nc.sync.dma_start(bf1_t[:], bf1_d.ap().rearrange("c p -> p c"))
            wzq_t = [wzq_a[:, d] for d in range(ND)]
            qx = [qx_a[:, d] for d in range(ND)]

            def xts(d, tcn):
                j = tcn % 4
                return xt_a[:, tcn // 4, d, j * 128:(j + 1) * 128]

            # ---- phase A: uT[d, s] = sum_a wzq[a,d] * qxT[a,s] ----
            # a-outer in two sb-halves (8 PSUM banks each, all m per half):
            # compute starts once wzq[a0]+qx[a0,sb0] land, and the sb0 ut
            # evictions (which gate the first scores blocks) overlap the
            # whole sb1 half.
            ut = [pb.tile([128, 1024], b16, tag=f"ut{m}", bufs=1,
                          name=f"ut{m}") for m in range(ND)]

            def phase_a(sb, ms, ups):
                for a in range(ND):
                    for m in ms:
                        nc.tensor.matmul(
                            ups[m][:],
                            wzq_t[a][:, m * 128:(m + 1) * 128],
                            qx[a][:, sb * 512:(sb + 1) * 512],
                            start=(a == 0), stop=(a == ND - 1))

            def evict_u(sb, ms, ups):
                # alternate vector / scalar so the eviction chain halves
                for m in ms:
                    dst = ut[m][:, sb * 512:(sb + 1) * 512]
                    if m % 2 == 0:
                        nc.vector.tensor_copy(dst, ups[m][:])
                    else:
                        nc.scalar.activation(dst, ups[m][:], AF.Copy)

            def phase_a_all():
                # m-groups of 4: the mm PSUM tag has 6 banks (den holds 2)
                for sb in range(2):
                    for mg in range(2):
                        ms = range(mg * 4, mg * 4 + 4)
                        ups = {m: psum() for m in ms}
                        phase_a(sb, ms, ups)
                        evict_u(sb, ms, ups)

            # phase-C weights prefetch into the same right pool (wzq/qx stay
            # live through the in-branch phase-A tail; fp8 weights fit all)
            wl_a = pr.tile([128, ND, D], b16, tag="wl", bufs=1)
            nc.sync.dma_start(wl_a[:], chunks(wvl_d, 0, ND, D))
            wf1_a = pr.tile([128, ND, D], f8, tag="wf1", bufs=1)
            nc.sync.dma_start(wf1_a[:], chunks(wff1_d, 0, ND, D))
            wf2_a = pr.tile([128, ND, D], f8, tag="wf2", bufs=1)
            nc.sync.dma_start(wf2_a[:], chunks(wff2_d, 0, ND, D))
            wvl_t = [wl_a[:, d] for d in range(ND)]

            attn = [pr.tile([128, 1024], b16, tag=f"at{d}", bufs=1,
                            name=f"at{d}") for d in range(ND)]

            def phase_b(parity):
                sb_slots = SB_SLOTS[parity]
                sb_pairs = SB_PAIRS[parity]
                DR = mybir.MatmulPerfMode.DoubleRow
                # normalized fp8 weights in DoubleRow pair layout; diagonal
                # blocks' pad columns [w:512] must be zero for paired reads
                et8 = {}
                for sb in (0, 1):
                    t8 = pb.tile([128, NT // 2, 2, 512], f8, tag=f"et8_{sb}",
                                 bufs=1, name=f"et8_{parity}_{sb}")
                    et8[sb] = t8
                    tc0 = sb_slots[sb][0]
                    nc.gpsimd.memset(t8[:, tc0 // 2, 0, 128:512], 0)
                    nc.gpsimd.memset(t8[:, tc0 // 2, 1, 256:512], 0)
                    nc.gpsimd.memset(t8[:, tc0 // 2 + 1, 0, 384:512], 0)

                # pass 1 per sb (big half first): scoresT -> exp, with the
                # diag mask folded into the scores PSUM as one extra matmul
                # (ident.T @ Lneg adds -30000 below the diagonal) and den
                # accumulated inline one block behind the scores matmuls.
                et = {}
                rbs = {}

                def pass1(sb):
                    slots = sb_slots[sb][::-1]   # descending: widest first
                    den_ps = psum_den()
                    for i, tcn in enumerate(slots):
                        w = _width(parity, sb, tcn)
                        diag = _is_diag(parity, sb, tcn)
                        scp = psum()
                        for d in range(ND):
                            nc.tensor.matmul(
                                scp[:, 0:w],
                                xts(d, tcn),
                                ut[d][:, sb * 512:sb * 512 + w],
                                start=(d == 0),
                                stop=(d == ND - 1 and not diag))
                        if diag:
                            nc.tensor.matmul(
                                scp[:, w - 128:w], ident_t, lneg_t,
                                start=False, stop=True)
                        if i > 0:
                            pt = slots[i - 1]
                            pw = _width(parity, sb, pt)
                            nc.tensor.matmul(
                                den_ps[:, 0:pw], ones_sq[:], et[(sb, pt)][:],
                                start=(i == 1), stop=False)
                        e = pb.tile([128, w], b16, tag=f"et{sb}_{tcn}",
                                    bufs=1, name=f"et{parity}_{sb}_{tcn}")
                        et[(sb, tcn)] = e
                        nc.scalar.activation(e[:], scp[:, 0:w], AF.Exp)
                    lt = slots[-1]
                    lw = _width(parity, sb, lt)
                    nc.tensor.matmul(
                        den_ps[:, 0:lw], ones_sq[:], et[(sb, lt)][:],
                        start=(len(slots) == 1), stop=True)
                    r = sp.tile([128, 512], f32, tag="rbs", bufs=2,
                                name=f"rbs{parity}_{sb}")
                    nc.vector.reciprocal(r[:], den_ps[:])
                    rbs[sb] = r

                def quantize(sb):
                    # et8 = et * (128/den), alternating vector/gpsimd
                    for i, tcn in enumerate(sb_slots[sb]):
                        w = _width(parity, sb, tcn)
                        eng = nc.vector if i % 2 == 0 else nc.gpsimd
                        eng.tensor_mul(
                            et8[sb][:, tcn // 2, tcn % 2, 0:w],
                            et[(sb, tcn)][:], rbs[sb][:, 0:w])

                def pass2(sb):
                    pairs = sb_pairs[sb][::-1]   # descending: widest first

                    def pw(k):
                        return _width(parity, sb, 2 * k + 1)

                    den2 = psum_den()
                    for i, k in enumerate(pairs):
                        nc.tensor.matmul(
                            den2[:, 0:pw(k)], ones8[:],
                            et8[sb][:, k, :, 0:pw(k)],
                            start=(i == 0), stop=(i == len(pairs) - 1),
                            perf_mode=DR)
                    r2 = sp.tile([128, 512], f32, tag="rbs2", bufs=2,
                                 name=f"rbs2{parity}_{sb}")
                    nc.vector.reciprocal(r2[:], den2[:])
                    for dc in range(ND):
                        axp = psum()
                        for i, k in enumerate(pairs):
                            nc.tensor.matmul(
                                axp[:, 0:pw(k)],
                                xn_a[:, 2 * k:2 * k + 2,
                                     dc * 128:(dc + 1) * 128],
                                et8[sb][:, k, :, 0:pw(k)],
                                start=(i == 0), stop=(i == len(pairs) - 1),
                                perf_mode=DR)
                        # PSUM reads are DVE-only (GpSimd can't touch PSUM)
                        nc.vector.tensor_mul(
                            attn[dc][:, sb * 512:(sb + 1) * 512],
                            axp[:], r2[:])

                pass1(0)
                quantize(0)     # DVE/Pool run under pass1(1)'s PE work
                pass1(1)
                quantize(1)
                pass2(0)
                pass2(1)

            # the entire phase A + B sits inside both branch bodies; the
            # branch is resolved right after warmup dispatch, overlapping
            # the startup DMA wait instead of stalling the PE mid-kernel
            with tc.If(par < 1) as cmp:
                phase_a_all()
                phase_b(0)
            with cmp.Else():
                phase_a_all()
                phase_b(1)

            # ---- free pB (ut/xt/xn/et); left pool for phase-C tiles ----
            esB.close()
            esC = es.enter_context(ExitStack())
            pc = esC.enter_context(tc.tile_pool(name="pC", bufs=1))

            x2f = [pc.tile([128, 1024], f32, tag=f"x2f{d}", bufs=1,
                           name=f"x2f{d}") for d in range(ND)]
            # Both FFN GEMMs run in fp8 DoubleRow. Scale chain: x2f carries
            # 32x (host scaled wvl/xq by 32); x2b = x2f/32 is true x2 in fp8;
            # w_ff1/w_ff2 are host-scaled by 32 into fp8's normal range; the
            # relu's scale=1/32 keeps ht exact; the final 32x output factor
            # is divided out on the host.
            x2b_a = pc.tile([128, ND, 1024], f8, tag="x2b", bufs=1)
            ht_a = pc.tile([128, ND, 1024], f8, tag="ht", bufs=1)

            # s2-halved pipeline: ff2(0)'s output adds (DVE) overlap ff1(1)'s
            # matmuls, so only ff2(1)'s tail is exposed past the last matmul
            def wvl_half(s2):
                cc = slice(s2 * 512, (s2 + 1) * 512)
                for oc in range(ND):
                    cps = psum()
                    for d in range(ND):
                        nc.tensor.matmul(
                            cps[:],
                            wvl_t[d][:, oc * 128:(oc + 1) * 128],
                            attn[d][:, cc],
                            start=(d == 0), stop=(d == ND - 1))
                    xqt = sp.tile([128, 512], f32, tag="xqt", bufs=4,
                                  name=f"xqt{oc}_{s2}")
                    nc.sync.dma_start(
                        xqt[:],
                        xq_d.ap()[oc * 128:(oc + 1) * 128, cc])
                    nc.vector.tensor_add(x2f[oc][:, cc], cps[:], xqt[:])
                    # scalar engine does the fp8 extraction; DVE stays free
                    # for the adds and any phase-B eviction spill-over
                    nc.scalar.activation(x2b_a[:, oc, cc], x2f[oc][:, cc],
                                         AF.Copy, scale=1.0 / 32)

            def ff1_half(s2):
                cc = slice(s2 * 512, (s2 + 1) * 512)
                for fc in range(ND):
                    cps = psum()
                    for d2 in range(0, ND, 2):
                        nc.tensor.matmul(
                            cps[:],
                            wf1_a[:, d2:d2 + 2, fc * 128:(fc + 1) * 128],
                            x2b_a[:, d2:d2 + 2, cc],
                            start=(d2 == 0), stop=(d2 == ND - 2),
                            perf_mode=mybir.MatmulPerfMode.DoubleRow)
                    nc.scalar.activation(ht_a[:, fc, cc], cps[:], AF.Relu,
                                         bias=bf1_t[:, fc:fc + 1],
                                         scale=1.0 / 32)

            def ff2_half(s2):
                cc = slice(s2 * 512, (s2 + 1) * 512)
                for oc in range(ND):
                    cps = psum()
                    for f2 in range(0, ND, 2):
                        nc.tensor.matmul(
                            cps[:],
                            wf2_a[:, f2:f2 + 2, oc * 128:(oc + 1) * 128],
                            ht_a[:, f2:f2 + 2, cc],
                            start=(f2 == 0), stop=(f2 == ND - 2),
                            perf_mode=mybir.MatmulPerfMode.DoubleRow)
                    # add in place into x2f (it has no later reader) and
                    # stream each oc out on alternating queues: no staging
                    # buffer reuse to wait on, no serialized descriptor chain
                    nc.vector.tensor_add(x2f[oc][:, cc], cps[:],
                                         x2f[oc][:, cc])
                    eng = nc.sync if oc % 2 == 0 else nc.gpsimd
                    eng.dma_start(
                        outT_d.ap()[oc * 128:(oc + 1) * 128, cc],
                        x2f[oc][:, cc])

            wvl_half(0)
            wvl_half(1)
            ff1_half(0)
            ff2_half(0)
            ff1_half(1)
            ff2_half(1)

    nc.compile()
    return nc


def _get_program():
    global _COMPILED
    if _COMPILED is None:
        _COMPILED = _build_program()
    return _COMPILED


def _p128(arr):
    """[c*128, C] -> [128, c*C]: the on-chip chunked layout, so device DMAs
    are contiguous per partition."""
    c = arr.shape[0] // 128
    return np.ascontiguousarray(
        arr.reshape(c, 128, -1).transpose(1, 0, 2).reshape(128, -1))


def _p128_xt(xT):
    """xT [D, S] -> [128, (cc, a, 512)]: 512-col chunks flat-contiguous so
    each chunk loads with full-bandwidth descriptors."""
    a = xT.reshape(ND, 128, 4, 512)            # [a, p, cc, n]
    return np.ascontiguousarray(
        a.transpose(1, 2, 0, 3).reshape(128, -1))


def kernel(x, wqkv, w_lin, b_lin, w_ff1, b_ff1, w_ff2, b_ff2):
    from concourse.bass_utils import run_bass_kernel_spmd

    x = np.asarray(x, np.float32)
    wqkv = np.asarray(wqkv, np.float32)
    Wq = wqkv[:, :D].astype(np.float64)
    Wk = wqkv[:, D:2 * D].astype(np.float64)
    Wv = wqkv[:, 2 * D:].astype(np.float64)

    F8 = ml_dtypes.float8_e4m3
    wzq = _p128(((Wq @ Wk.T) / 2.0).astype(BF16))   # [a, d] natural layout
    # phase C carries a 32x scale (wvl, xq) so x2b = x2f/32 is exact x2;
    # both FFN weights are scaled by 32 into fp8's normal range; the final
    # 32x on the output is divided out below
    wvl = _p128((Wv @ np.asarray(w_lin, np.float64) * 32.0).astype(BF16))
    wff1 = _p128((np.asarray(w_ff1, np.float32) * 32.0).astype(F8))
    wff2 = _p128((np.asarray(w_ff2, np.float32) * 32.0).astype(F8))
    # ident | Lneg: the anti-causal diag mask as a PE-side score bias
    lneg = np.where(np.arange(128)[:, None] < np.arange(128)[None, :],
                    np.float32(-30000.0), np.float32(0.0))
    tri = np.concatenate([np.eye(128, dtype=np.float32), lneg],
                         axis=1).astype(BF16)

    in_maps = []
    qcols_by_parity = {
        0: np.r_[0:512, 1536:2048],
        1: np.r_[512:1536],
    }
    b_lin = np.asarray(b_lin, np.float32)
    b_ff1 = np.asarray(b_ff1, np.float32)
    b_ff2 = np.asarray(b_ff2, np.float32)
    bf1 = np.ascontiguousarray(b_ff1.reshape(ND, 128))
    for c in range(NCORES):
        b, h = c // 2, c % 2
        xT32 = np.ascontiguousarray(x[b].T)               # [D, S] f32
        qcols = qcols_by_parity[h]
        qxT32 = np.ascontiguousarray(xT32[:, qcols])      # [D, 1024]
        in_maps.append({
            "xT": _p128_xt(xT32.astype(BF16)),
            "xn": _p128(x[b].astype(F8)),                 # [S, D] natural
            "qxT": _p128(qxT32.astype(BF16)),
            "xq": (qxT32 + b_lin[:, None]) * 32.0,        # b_lin folded in
            "wzq": wzq,
            "wvl": wvl,
            "wff1": wff1,
            "wff2": wff2,
            "tri": tri,
            "bf1": bf1,
            "par": np.full((1, 1), h, np.uint32),
        })

    global _LAST_IN_MAPS
    _LAST_IN_MAPS = in_maps
    nc = _get_program()
    res = run_bass_kernel_spmd(nc, in_maps, core_ids=list(range(NCORES)))

    out = np.empty((B, S, D), np.float32)
    for c in range(NCORES):
        b, h = c // 2, c % 2
        ol = res.results[c]["outT"].T / 32.0              # [1024 s, D]
        if h == 0:
            out[b, 0:512] = ol[:512]
            out[b, 1536:2048] = ol[512:]
        else:
            out[b, 512:1536] = ol
    out += b_ff2[None, None, :]
    return out



# revision 20
# speedup vs baseline: 1.0813x; 1.0813x over previous
"""Trainium2 Bass kernel for nn_MultiHeadAttention_48395691492077.

Reference (B=4, S=2048, D=1024, single head, anti-causal triu mask):
    qkv = x @ wqkv; q,k,v = split(qkv)
    scores = triu(q @ k^T / sqrt(B));  masked softmax over keys t >= s
    x2  = softmax(scores) @ v @ w_lin + b_lin + x
    out = relu(x2 @ w_ff1 + b_ff1) @ w_ff2 + b_ff2 + x2

Sharding: 8 cores = 4 batches x 2 query-halves. Each core computes
attention + MLP for its own 1024 queries against the full 2048-key
sequence of its batch. The program is identical on all cores (SPMD);
per-core differences (which queries, which mask pattern) are carried in
the input data plus one branch on the query-parity register.

Device algebra (transposed; no on-chip transposes, no K/V projections):
    uT = wzq^T.T @ qxT            with wzq = (Wq @ Wk^T)/2  (host-fused)
    scoresT[t,s] = sum_d xT[d,t] * uT[d,s]     (keys are raw x!)
    expT = exp(scoresT); diagonal 128-strips *= lower-tri mask
    den[s]/128 broadcast = (ones/128).T @ expT (PE, inlined into pass 1)
    et8 = fp8(expT * 128/den)   (row-stochastic weights x128, fp8 range-safe)
    den2[s] = ones8.T @ et8     (fp8 DoubleRow; renormalizes the fp8
          quantization so attention stays exactly row-stochastic)
    H^T[d,s] = xn8[t,d].T @ et8 (A@X in fp8 DoubleRow; V proj deferred)
    attnT = H^T / den2
    x2T = wvl.T @ attnT + (xT + b_lin)  with wvl = Wv @ w_lin (host-fused:
          A@(X@Wv)@w_lin == (A@X)@(Wv@w_lin) by associativity)
    hT = relu(w_ff1.T @ x2T + b_ff1)
    outT = w_ff2.T @ hT + x2T               (+ b_ff2 added on host)
Blocks crossing the anti-causal diagonal use exact matmul widths
(128/256/384) instead of full 512; PSUM accumulation runs widest-first
so every column is initialized by the start=True matmul. A@X pairs
adjacent t-chunks for DoubleRow; diagonal blocks' tail columns are
zero-padded once so paired reads stay correct.
Matmul inputs are bf16 (fp32 PSUM accumulation) except A@X/FFN (fp8
DoubleRow); residuals are fp32. ff2 adds in place into x2f and streams
per-oc output DMAs on alternating queues to keep the tail short.
"""

import numpy as np
import ml_dtypes

B, S, D = 4, 2048, 1024
NCORES = 8
BF16 = ml_dtypes.bfloat16

NT = S // 128            # 16 t-chunks
ND = D // 128            # 8 chunks of 128 along any D-sized dim

# global query-column starts of the four 256-query groups per parity.
# Together they tile the same query sets as the old 512-col halves, so the
# host-side qcols mapping is unchanged. 256-col groups give the phase-B
# pipeline four units: quantize(g) hides under pass1(g+1)/A@X(g-1) PE work.
GRP_STARTS = {0: (0, 256, 1536, 1792), 1: (512, 768, 1024, 1280)}
GW = 256                 # group width (queries per group)


def _gslots(parity, g):
    return list(range(GRP_STARTS[parity][g] // 128, NT))


def _gpairs(parity, g):
    return list(range(GRP_STARTS[parity][g] // 256, NT // 2))


def _width(parity, g, tc):
    """Valid column count of block (g, tc): cols [0, w) of the 256."""
    return min(GW, 128 * tc - GRP_STARTS[parity][g] + 128)


def _is_diag(parity, g, tc):
    """Block whose last 128 columns lie on the anti-causal diagonal."""
    return tc - GRP_STARTS[parity][g] // 128 < 2


_COMPILED = None
_LAST_IN_MAPS = None


def _build_program():
    from contextlib import ExitStack
    import concourse.bacc as bacc
    import concourse.mybir as mybir
    import concourse.tile as tile

    f32 = mybir.dt.float32
    b16 = mybir.dt.bfloat16
    AF = mybir.ActivationFunctionType

    nc = bacc.Bacc("TRN2", target_bir_lowering=False, debug=False,
                   num_devices=NCORES)

    # all big inputs arrive pre-arranged on the host into the on-chip
    # [128, chunk, free] layout so every DMA is contiguous per partition
    f8 = mybir.dt.float8e4
    xT_d = nc.dram_tensor("xT", [128, ND * S], b16, kind="ExternalInput")
    xn_d = nc.dram_tensor("xn", [128, NT * D], f8, kind="ExternalInput")
    qxT_d = nc.dram_tensor("qxT", [128, ND * 1024], b16, kind="ExternalInput")
    xq_d = nc.dram_tensor("xq", [D, 1024], f32, kind="ExternalInput")
    wzq_d = nc.dram_tensor("wzq", [128, ND * D], b16, kind="ExternalInput")
    wvl_d = nc.dram_tensor("wvl", [128, ND * D], b16, kind="ExternalInput")
    wff1_d = nc.dram_tensor("wff1", [128, ND * D], f8, kind="ExternalInput")
    wff2_d = nc.dram_tensor("wff2", [128, ND * D], f8, kind="ExternalInput")
    # ident | Lneg: identity and the strictly-lower -30000 mask-bias, both
    # bf16; the diag mask is applied as one extra PE matmul into the scores
    # PSUM (ident.T @ Lneg adds -30000 below the diagonal) instead of a
    # post-exp vector multiply
    tri_d = nc.dram_tensor("tri", [128, 256], b16, kind="ExternalInput")
    par_d = nc.dram_tensor("par", [1, 1], mybir.dt.uint32, kind="ExternalInput")
    bf1_d = nc.dram_tensor("bf1", [ND, 128], f32, kind="ExternalInput")
    # bf16 output halves the exit DMA; host upcasts (costs ~2e-4 rel err)
    outT_d = nc.dram_tensor("outT", [D, 1024], b16, kind="ExternalOutput")

    with tile.TileContext(nc) as tc:
        es = ExitStack()
        with es:
            pp = es.enter_context(tc.tile_pool(name="persist", bufs=1))
            sp = es.enter_context(tc.tile_pool(name="stream", bufs=2))
            ps = es.enter_context(
                tc.tile_pool(name="ps", bufs=8, space="PSUM"))
            esB = es.enter_context(ExitStack())
            pb = esB.enter_context(tc.tile_pool(name="pB", bufs=1))
            pr = es.enter_context(tc.tile_pool(name="pAC", bufs=1,
                                               side="right"))

            def psum():
                t = ps.tile([128, 512], f32, tag="mm", bufs=6, name="mmps")
                return t

            def psum_den():
                # den/den2 banks stay live across many mm-tag rotations
                return ps.tile([128, 512], f32, tag="den", bufs=2, name="denps")

            # ---- constants ----
            # load the parity register up front so every engine sequencer
            # resolves it during the startup DMA wait, not at the branch
            par_regs = nc.alloc_registers("par_regs")
            nc.regs_load(par_regs, par_d.ap()[0:1, 0:1])
            par = nc.snap(par_regs, donate=True, min_val=0, max_val=1)

            # den accumulates (ones/128).T @ et so rbs = recip gives 128/den,
            # folding the fp8 weight scale (x128) into the reciprocal for free
            ones_sq = pp.tile([128, 128], b16, tag="ones_sq", bufs=1)
            nc.vector.memset(ones_sq[:], 1.0 / 128)
            ones8 = pp.tile([128, 2, 128], f8, tag="ones8", bufs=1)
            nc.vector.memset(ones8[:], 1.0)
            tri_t = pp.tile([128, 256], b16, tag="tri", bufs=1)
            ident_t = tri_t[:, 0:128]
            lneg_t = tri_t[:, 128:256]
            # warm the PE HAM clock-gate while the first input DMAs land
            wups = psum()
            for i in range(16):
                nc.tensor.matmul(wups[:, 0:128], ones_sq[:], ones_sq[:],
                                 start=(i == 0), stop=(i == 15))

            # ---- input loads (arrival-ordered for phase-A pipelining).
            # Descriptor generation serializes per issuing queue (~0.7us per
            # dma_start), so the early loads fan out across engine queues.
            def chunks(dram, c0, c1, width):
                return dram.ap()[:, c0 * width:c1 * width].rearrange(
                    "p (c n) -> p c n", n=width)

            wzq_a = pr.tile([128, ND, D], b16, tag="wzq", bufs=1)
            qx_a = pr.tile([128, ND, 1024], b16, tag="qx", bufs=1)
            # single sync queue = priority order at HBM; 4-chunk granules
            # beat the ~0.65us/descriptor issue rate that single-a granules
            # paid, so phase A never catches up with the arrivals
            qxv = qxT_d.ap().rearrange("p (c n) -> p c n", n=1024)
            for h in range(2):
                aa = slice(h * 4, h * 4 + 4)
                nc.sync.dma_start(qx_a[:, aa, 0:512], qxv[:, aa, 0:512])
                nc.sync.dma_start(wzq_a[:, aa], chunks(wzq_d, h * 4, h * 4 + 4, D))
            nc.sync.dma_start(qx_a[:, :, 512:1024], qxv[:, :, 512:1024])
            # xT feeds the scores pass; host interleaves it so each 512-col
            # chunk is flat-contiguous (full-bandwidth descriptors). Pass 1
            # runs descending from tc15, so load high chunks first.
            xt_a = pb.tile([128, 4, ND, 512], b16, tag="xt", bufs=1)
            for cc in (3, 2, 1, 0):
                nc.sync.dma_start(
                    xt_a[:, cc],
                    xT_d.ap()[:, cc * 4096:(cc + 1) * 4096]
                    .rearrange("p (a n) -> p a n", n=512))
            # x natural layout [t, d] in fp8 feeds the A@X DoubleRow pass
            xn_a = pb.tile([128, NT, D], f8, tag="xn", bufs=1)
            nc.sync.dma_start(xn_a[:], chunks(xn_d, 0, NT, D))
            nc.sync.dma_start(tri_t[:], tri_d.ap())
            # b_ff1 laid out [128, ND]: bias column fc serves f-chunk fc
            bf1_t = pp.tile([128, ND], f32, tag="bf1", bufs=1)
            nc.sync.dma_start(bf1_t[:], bf1_d.ap().rearrange("c p -> p c"))
            wzq_t = [wzq_a[:, d] for d in range(ND)]
            qx = [qx_a[:, d] for d in range(ND)]

            def xts(d, tcn):
                j = tcn % 4
                return xt_a[:, tcn // 4, d, j * 128:(j + 1) * 128]

            # ---- phase A: uT[d, s] = sum_a wzq[a,d] * qxT[a,s] ----
            # a-outer in two sb-halves (8 PSUM banks each, all m per half):
            # compute starts once wzq[a0]+qx[a0,sb0] land, and the sb0 ut
            # evictions (which gate the first scores blocks) overlap the
            # whole sb1 half.
            ut = [pb.tile([128, 1024], b16, tag=f"ut{m}", bufs=1,
                          name=f"ut{m}") for m in range(ND)]

            def phase_a(sb, ms, ups):
                for a in range(ND):
                    for m in ms:
                        nc.tensor.matmul(
                            ups[m][:],
                            wzq_t[a][:, m * 128:(m + 1) * 128],
                            qx[a][:, sb * 512:(sb + 1) * 512],
                            start=(a == 0), stop=(a == ND - 1))

            def evict_u(sb, ms, ups):
                # alternate vector / scalar so the eviction chain halves
                for m in ms:
                    dst = ut[m][:, sb * 512:(sb + 1) * 512]
                    if m % 2 == 0:
                        nc.vector.tensor_copy(dst, ups[m][:])
                    else:
                        nc.scalar.activation(dst, ups[m][:], AF.Copy)

            def phase_a_all():
                # m-groups of 4: the mm PSUM tag has 6 banks (den holds 2)
                for sb in range(2):
                    for mg in range(2):
                        ms = range(mg * 4, mg * 4 + 4)
                        ups = {m: psum() for m in ms}
                        phase_a(sb, ms, ups)
                        evict_u(sb, ms, ups)

            # phase-C weights prefetch into the same right pool (wzq/qx stay
            # live through the in-branch phase-A tail; fp8 weights fit all)
            wl_a = pr.tile([128, ND, D], b16, tag="wl", bufs=1)
            nc.sync.dma_start(wl_a[:], chunks(wvl_d, 0, ND, D))
            wf1_a = pr.tile([128, ND, D], f8, tag="wf1", bufs=1)
            nc.sync.dma_start(wf1_a[:], chunks(wff1_d, 0, ND, D))
            wf2_a = pr.tile([128, ND, D], f8, tag="wf2", bufs=1)
            nc.sync.dma_start(wf2_a[:], chunks(wff2_d, 0, ND, D))
            wvl_t = [wl_a[:, d] for d in range(ND)]

            attn = [pr.tile([128, 1024], b16, tag=f"at{d}", bufs=1,
                            name=f"at{d}") for d in range(ND)]

            def phase_b(parity):
                DR = mybir.MatmulPerfMode.DoubleRow
                # normalized fp8 weights in DoubleRow pair layout; the first
                # (128-wide) block's pad columns [128:256] must be zero for
                # paired reads
                et8 = {}
                for g in range(4):
                    t8 = pb.tile([128, NT // 2, 2, GW], f8, tag=f"et8_{g}",
                                 bufs=1, name=f"et8_{parity}_{g}")
                    et8[g] = t8
                    k0 = GRP_STARTS[parity][g] // 256
                    nc.gpsimd.memset(t8[:, k0, 0, 128:GW], 0)

                # pass 1 per group: scoresT -> exp, with the diag mask folded
                # into the scores PSUM as one extra matmul (ident.T @ Lneg
                # adds -30000 below the diagonal) and den accumulated inline
                # one block behind the scores matmuls.
                et = {}
                rbs = {}

                def pass1(g):
                    g0 = GRP_STARTS[parity][g]
                    slots = _gslots(parity, g)[::-1]   # widest first
                    den_ps = psum_den()
                    for i, tcn in enumerate(slots):
                        w = _width(parity, g, tcn)
                        diag = _is_diag(parity, g, tcn)
                        scp = psum()
                        for d in range(ND):
                            nc.tensor.matmul(
                                scp[:, 0:w],
                                xts(d, tcn),
                                ut[d][:, g * GW:g * GW + w],
                                start=(d == 0),
                                stop=(d == ND - 1 and not diag))
                        if diag:
                            nc.tensor.matmul(
                                scp[:, w - 128:w], ident_t, lneg_t,
                                start=False, stop=True)
                        if i > 0:
                            pt = slots[i - 1]
                            pw = _width(parity, g, pt)
                            nc.tensor.matmul(
                                den_ps[:, 0:pw], ones_sq[:], et[(g, pt)][:],
                                start=(i == 1), stop=False)
                        e = pb.tile([128, w], b16, tag=f"et{g}_{tcn}",
                                    bufs=1, name=f"et{parity}_{g}_{tcn}")
                        et[(g, tcn)] = e
                        nc.scalar.activation(e[:], scp[:, 0:w], AF.Exp)
                    lt = slots[-1]
                    lw = _width(parity, g, lt)
                    nc.tensor.matmul(
                        den_ps[:, 0:lw], ones_sq[:], et[(g, lt)][:],
                        start=(len(slots) == 1), stop=True)
                    r = sp.tile([128, GW], f32, tag="rbs", bufs=2,
                                name=f"rbs{parity}_{g}")
                    nc.vector.reciprocal_approx_fast(r[:], den_ps[:, 0:GW])
                    rbs[g] = r

                def quantize(g):
                    # et8 = et * (128/den), alternating vector/gpsimd
                    for i, tcn in enumerate(_gslots(parity, g)):
                        w = _width(parity, g, tcn)
                        eng = nc.vector if i % 2 == 0 else nc.gpsimd
                        eng.tensor_mul(
                            et8[g][:, tcn // 2, tcn % 2, 0:w],
                            et[(g, tcn)][:], rbs[g][:, 0:w])

                def pass2(g):
                    pairs = _gpairs(parity, g)[::-1]   # widest first

                    def pw(k):
                        return _width(parity, g, 2 * k + 1)

                    den2 = psum_den()
                    for i, k in enumerate(pairs):
                        nc.tensor.matmul(
                            den2[:, 0:pw(k)], ones8[:],
                            et8[g][:, k, :, 0:pw(k)],
                            start=(i == 0), stop=(i == len(pairs) - 1),
                            perf_mode=DR)
                    r2 = sp.tile([128, GW], f32, tag="rbs2", bufs=2,
                                 name=f"rbs2{parity}_{g}")
                    nc.vector.reciprocal_approx_fast(r2[:], den2[:, 0:GW])
                    for dc in range(ND):
                        axp = psum()
                        for i, k in enumerate(pairs):
                            nc.tensor.matmul(
                                axp[:, 0:pw(k)],
                                xn_a[:, 2 * k:2 * k + 2,
                                     dc * 128:(dc + 1) * 128],
                                et8[g][:, k, :, 0:pw(k)],
                                start=(i == 0), stop=(i == len(pairs) - 1),
                                perf_mode=DR)
                        # PSUM reads are DVE-only (GpSimd can't touch PSUM)
                        nc.vector.tensor_mul(
                            attn[dc][:, g * GW:(g + 1) * GW],
                            axp[:, 0:GW], r2[:])

                # software pipeline: quantize(g) (DVE+Pool) hides under
                # pass1(g+1) and A@X(g-1) PE work; recips are approx_fast
                pass1(0)
                quantize(0)
                pass1(1)
                quantize(1)
                pass2(0)
                pass1(2)
                quantize(2)
                pass2(1)
                pass1(3)
                quantize(3)
                pass2(2)
                pass2(3)

            # the entire phase A + B sits inside both branch bodies; the
            # branch is resolved right after warmup dispatch, overlapping
            # the startup DMA wait instead of stalling the PE mid-kernel
            with tc.If(par < 1) as cmp:
                phase_a_all()
                phase_b(0)
            with cmp.Else():
                phase_a_all()
                phase_b(1)

            # ---- free pB (ut/xt/xn/et); left pool for phase-C tiles ----
            esB.close()
            esC = es.enter_context(ExitStack())
            pc = esC.enter_context(tc.tile_pool(name="pC", bufs=1))

            x2f = [pc.tile([128, 1024], f32, tag=f"x2f{d}", bufs=1,
                           name=f"x2f{d}") for d in range(ND)]
            # Both FFN GEMMs run in fp8 DoubleRow. Scale chain: x2f carries
            # 32x (host scaled wvl/xq by 32); x2b = x2f/32 is true x2 in fp8;
            # w_ff1/w_ff2 are host-scaled by 32 into fp8's normal range; the
            # relu's scale=1/32 keeps ht exact; the final 32x output factor
            # is divided out on the host.
            x2b_a = pc.tile([128, ND, 1024], f8, tag="x2b", bufs=1)
            ht_a = pc.tile([128, ND, 1024], f8, tag="ht", bufs=1)

            # s2-halved pipeline: ff2(0)'s output adds (DVE) overlap ff1(1)'s
            # matmuls, so only ff2(1)'s tail is exposed past the last matmul
            def wvl_half(s2):
                cc = slice(s2 * 512, (s2 + 1) * 512)
                for oc in range(ND):
                    cps = psum()
                    for d in range(ND):
                        nc.tensor.matmul(
                            cps[:],
                            wvl_t[d][:, oc * 128:(oc + 1) * 128],
                            attn[d][:, cc],
                            start=(d == 0), stop=(d == ND - 1))
                    xqt = sp.tile([128, 512], f32, tag="xqt", bufs=4,
                                  name=f"xqt{oc}_{s2}")
                    nc.sync.dma_start(
                        xqt[:],
                        xq_d.ap()[oc * 128:(oc + 1) * 128, cc])
                    nc.vector.tensor_add(x2f[oc][:, cc], cps[:], xqt[:])
                    # scalar engine does the fp8 extraction; DVE stays free
                    # for the adds and any phase-B eviction spill-over
                    nc.scalar.activation(x2b_a[:, oc, cc], x2f[oc][:, cc],
                                         AF.Copy, scale=1.0 / 32)

            def ff1_half(s2):
                cc = slice(s2 * 512, (s2 + 1) * 512)
                for fc in range(ND):
                    cps = psum()
                    for d2 in range(0, ND, 2):
                        nc.tensor.matmul(
                            cps[:],
                            wf1_a[:, d2:d2 + 2, fc * 128:(fc + 1) * 128],
                            x2b_a[:, d2:d2 + 2, cc],
                            start=(d2 == 0), stop=(d2 == ND - 2),
                            perf_mode=mybir.MatmulPerfMode.DoubleRow)
                    nc.scalar.activation(ht_a[:, fc, cc], cps[:], AF.Relu,
                                         bias=bf1_t[:, fc:fc + 1],
                                         scale=1.0 / 32)

            def ff2_half(s2):
                cc = slice(s2 * 512, (s2 + 1) * 512)
                for oc in range(ND):
                    cps = psum()
                    for f2 in range(0, ND, 2):
                        nc.tensor.matmul(
                            cps[:],
                            wf2_a[:, f2:f2 + 2, oc * 128:(oc + 1) * 128],
                            ht_a[:, f2:f2 + 2, cc],
                            start=(f2 == 0), stop=(f2 == ND - 2),
                            perf_mode=mybir.MatmulPerfMode.DoubleRow)
                    # bf16 staging tile (bufs=8: no reuse wait within a
                    # half) and per-oc DMA on alternating queues: no
                    # serialized descriptor chain, half the exit bytes
                    ot = sp.tile([128, 512], b16, tag="ot", bufs=8,
                                 name=f"ot{oc}_{s2}")
                    nc.vector.tensor_add(ot[:], cps[:], x2f[oc][:, cc])
                    eng = nc.sync if oc % 2 == 0 else nc.gpsimd
                    eng.dma_start(
                        outT_d.ap()[oc * 128:(oc + 1) * 128, cc], ot[:])

            wvl_half(0)
            wvl_half(1)
            ff1_half(0)
            ff2_half(0)
            ff1_half(1)
            ff2_half(1)

    nc.compile()
    return nc


def _get_program():
    global _COMPILED
    if _COMPILED is None:
        _COMPILED = _build_program()
    return _COMPILED


def _p128(arr):
    """[c*128, C] -> [128, c*C]: the on-chip chunked layout, so device DMAs
    are contiguous per partition."""
    c = arr.shape[0] // 128
    return np.ascontiguousarray(
        arr.reshape(c, 128, -1).transpose(1, 0, 2).reshape(128, -1))


def _p128_xt(xT):
    """xT [D, S] -> [128, (cc, a, 512)]: 512-col chunks flat-contiguous so
    each chunk loads with full-bandwidth descriptors."""
    a = xT.reshape(ND, 128, 4, 512)            # [a, p, cc, n]
    return np.ascontiguousarray(
        a.transpose(1, 2, 0, 3).reshape(128, -1))


def kernel(x, wqkv, w_lin, b_lin, w_ff1, b_ff1, w_ff2, b_ff2):
    from concourse.bass_utils import run_bass_kernel_spmd

    x = np.asarray(x, np.float32)
    wqkv = np.asarray(wqkv, np.float32)
    Wq = wqkv[:, :D].astype(np.float64)
    Wk = wqkv[:, D:2 * D].astype(np.float64)
    Wv = wqkv[:, 2 * D:].astype(np.float64)

    F8 = ml_dtypes.float8_e4m3
    wzq = _p128(((Wq @ Wk.T) / 2.0).astype(BF16))   # [a, d] natural layout
    # phase C carries a 32x scale (wvl, xq) so x2b = x2f/32 is exact x2;
    # both FFN weights are scaled by 32 into fp8's normal range; the final
    # 32x on the output is divided out below
    wvl = _p128((Wv @ np.asarray(w_lin, np.float64) * 32.0).astype(BF16))
    wff1 = _p128((np.asarray(w_ff1, np.float32) * 32.0).astype(F8))
    wff2 = _p128((np.asarray(w_ff2, np.float32) * 32.0).astype(F8))
    # ident | Lneg: the anti-causal diag mask as a PE-side score bias
    lneg = np.where(np.arange(128)[:, None] < np.arange(128)[None, :],
                    np.float32(-30000.0), np.float32(0.0))
    tri = np.concatenate([np.eye(128, dtype=np.float32), lneg],
                         axis=1).astype(BF16)

    in_maps = []
    qcols_by_parity = {
        0: np.r_[0:512, 1536:2048],
        1: np.r_[512:1536],
    }
    b_lin = np.asarray(b_lin, np.float32)
    b_ff1 = np.asarray(b_ff1, np.float32)
    b_ff2 = np.asarray(b_ff2, np.float32)
    bf1 = np.ascontiguousarray(b_ff1.reshape(ND, 128))
    for c in range(NCORES):
        b, h = c // 2, c % 2
        xT32 = np.ascontiguousarray(x[b].T)               # [D, S] f32
        qcols = qcols_by_parity[h]
        qxT32 = np.ascontiguousarray(xT32[:, qcols])      # [D, 1024]
        in_maps.append({
            "xT": _p128_xt(xT32.astype(BF16)),
            "xn": _p128(x[b].astype(F8)),                 # [S, D] natural
            "qxT": _p128(qxT32.astype(BF16)),
            "xq": (qxT32 + b_lin[:, None]) * 32.0,        # b_lin folded in
            "wzq": wzq,
            "wvl": wvl,
            "wff1": wff1,
            "wff2": wff2,
            "tri": tri,
            "bf1": bf1,
            "par": np.full((1, 1), h, np.uint32),
        })

    global _LAST_IN_MAPS
    _LAST_IN_MAPS = in_maps
    nc = _get_program()
    res = run_bass_kernel_spmd(nc, in_maps, core_ids=list(range(NCORES)))

    out = np.empty((B, S, D), np.float32)
    for c in range(NCORES):
        b, h = c // 2, c % 2
        ol = np.asarray(res.results[c]["outT"],
                        np.float32).T / 32.0              # [1024 s, D]
        if h == 0:
            out[b, 0:512] = ol[:512]
            out[b, 1536:2048] = ol[512:]
        else:
            out[b, 512:1536] = ol
    out += b_ff2[None, None, :]
    return out



# revision 25
# speedup vs baseline: 1.1129x; 1.0293x over previous
"""Trainium2 Bass kernel for nn_MultiHeadAttention_48395691492077.

Reference (B=4, S=2048, D=1024, single head, anti-causal triu mask):
    qkv = x @ wqkv; q,k,v = split(qkv)
    scores = triu(q @ k^T / sqrt(B));  masked softmax over keys t >= s
    x2  = softmax(scores) @ v @ w_lin + b_lin + x
    out = relu(x2 @ w_ff1 + b_ff1) @ w_ff2 + b_ff2 + x2

Sharding: 8 cores = 4 batches x 2 query-halves. Each core computes
attention + MLP for its own 1024 queries against the full 2048-key
sequence of its batch. The program is identical on all cores (SPMD);
per-core differences (which queries, which mask pattern) are carried in
the input data plus one branch on the query-parity register.

Device algebra (transposed; no on-chip transposes, no K/V projections):
    uT = wzq^T.T @ qxT            with wzq = (Wq @ Wk^T)/2  (host-fused)
    scoresT[t,s] = sum_d xT[d,t] * uT[d,s]     (keys are raw x!)
    expT = exp(scoresT); diagonal 128-strips *= lower-tri mask
    den[s]/128 broadcast = (ones/128).T @ expT (PE, inlined into pass 1)
    et8 = fp8(expT * 128/den)   (row-stochastic weights x128, fp8 range-safe)
    den2[s] = ones8.T @ et8     (fp8 DoubleRow; renormalizes the fp8
          quantization so attention stays exactly row-stochastic)
    H^T[d,s] = xn8[t,d].T @ et8 (A@X in fp8 DoubleRow; V proj deferred)
    attnT = H^T / den2
    x2T = wvl.T @ attnT + (xT + b_lin)  with wvl = Wv @ w_lin (host-fused:
          A@(X@Wv)@w_lin == (A@X)@(Wv@w_lin) by associativity)
    hT = relu(w_ff1.T @ x2T + b_ff1)
    outT = w_ff2.T @ hT + x2T               (+ b_ff2 added on host)
Blocks crossing the anti-causal diagonal use exact matmul widths
(128/256/384) instead of full 512; PSUM accumulation runs widest-first
so every column is initialized by the start=True matmul. A@X pairs
adjacent t-chunks for DoubleRow; diagonal blocks' tail columns are
zero-padded once so paired reads stay correct.
Matmul inputs are bf16 (fp32 PSUM accumulation) except A@X/FFN (fp8
DoubleRow); residuals are fp32. ff2 adds in place into x2f and streams
per-oc output DMAs on alternating queues to keep the tail short.
"""

import numpy as np
import ml_dtypes

B, S, D = 4, 2048, 1024
NCORES = 8
BF16 = ml_dtypes.bfloat16

NT = S // 128            # 16 t-chunks
ND = D // 128            # 8 chunks of 128 along any D-sized dim

# global query-column starts of the four 256-query groups per parity.
# Together they tile the same query sets as the old 512-col halves, so the
# host-side qcols mapping is unchanged. 256-col groups give the phase-B
# pipeline four units: quantize(g) hides under pass1(g+1)/A@X(g-1) PE work.
GRP_STARTS = {0: (0, 256, 1536, 1792), 1: (512, 768, 1024, 1280)}
GW = 256                 # group width (queries per group)
# 512-query halves (pairs of groups) used by den2/A@X so the DoubleRow
# matmuls run at FD 512 (FD 256 is LDWEIGHTS-bound: 182ns vs 110ns/pair).
# Order per parity: the half whose quantize finishes first goes first.
HALF_STARTS = {0: (0, 1536), 1: (512, 1024)}
HALF_ORDER = {0: (1, 0), 1: (0, 1)}


def _gslots(parity, g):
    return list(range(GRP_STARTS[parity][g] // 128, NT))


def _width(parity, g, tc):
    """Valid column count of block (g, tc): cols [0, w) of the 256."""
    return min(GW, 128 * tc - GRP_STARTS[parity][g] + 128)


def _is_diag(parity, g, tc):
    """Block whose last 128 columns lie on the anti-causal diagonal."""
    return tc - GRP_STARTS[parity][g] // 128 < 2


def _hpairs(parity, h):
    return list(range(HALF_STARTS[parity][h] // 256, NT // 2))


def _hwidth(parity, h, tc):
    return min(512, 128 * tc - HALF_STARTS[parity][h] + 128)


_COMPILED = None
_LAST_IN_MAPS = None


def _build_program():
    from contextlib import ExitStack
    import concourse.bacc as bacc
    import concourse.mybir as mybir
    import concourse.tile as tile

    f32 = mybir.dt.float32
    b16 = mybir.dt.bfloat16
    AF = mybir.ActivationFunctionType

    nc = bacc.Bacc("TRN2", target_bir_lowering=False, debug=False,
                   num_devices=NCORES)

    # all big inputs arrive pre-arranged on the host into the on-chip
    # [128, chunk, free] layout so every DMA is contiguous per partition
    f8 = mybir.dt.float8e4
    xT_d = nc.dram_tensor("xT", [128, ND * S], b16, kind="ExternalInput")
    xn_d = nc.dram_tensor("xn", [128, NT * D], f8, kind="ExternalInput")
    qxT_d = nc.dram_tensor("qxT", [128, ND * 1024], b16, kind="ExternalInput")
    xq_d = nc.dram_tensor("xq", [D, 1024], f32, kind="ExternalInput")
    wzq_d = nc.dram_tensor("wzq", [128, ND * D], b16, kind="ExternalInput")
    wvl_d = nc.dram_tensor("wvl", [128, ND * D], b16, kind="ExternalInput")
    wff1_d = nc.dram_tensor("wff1", [128, ND * D], f8, kind="ExternalInput")
    wff2_d = nc.dram_tensor("wff2", [128, ND * D], f8, kind="ExternalInput")
    # ident | Lneg: identity and the strictly-lower -30000 mask-bias, both
    # bf16; the diag mask is applied as one extra PE matmul into the scores
    # PSUM (ident.T @ Lneg adds -30000 below the diagonal) instead of a
    # post-exp vector multiply
    tri_d = nc.dram_tensor("tri", [128, 256], b16, kind="ExternalInput")
    par_d = nc.dram_tensor("par", [1, 1], mybir.dt.uint32, kind="ExternalInput")
    bf1_d = nc.dram_tensor("bf1", [ND, 128], f32, kind="ExternalInput")
    # bf16 output halves the exit DMA; host upcasts (costs ~2e-4 rel err)
    outT_d = nc.dram_tensor("outT", [D, 1024], b16, kind="ExternalOutput")

    with tile.TileContext(nc) as tc:
        es = ExitStack()
        with es:
            pp = es.enter_context(tc.tile_pool(name="persist", bufs=1))
            sp = es.enter_context(tc.tile_pool(name="stream", bufs=2))
            ps = es.enter_context(
                tc.tile_pool(name="ps", bufs=8, space="PSUM"))
            esB = es.enter_context(ExitStack())
            pb = esB.enter_context(tc.tile_pool(name="pB", bufs=1))
            pr = es.enter_context(tc.tile_pool(name="pAC", bufs=1,
                                               side="right"))

            def psum():
                t = ps.tile([128, 512], f32, tag="mm", bufs=6, name="mmps")
                return t

            def psum_den():
                # den/den2 banks stay live across many mm-tag rotations
                return ps.tile([128, 512], f32, tag="den", bufs=2, name="denps")

            # ---- constants ----
            # load the parity register up front so every engine sequencer
            # resolves it during the startup DMA wait, not at the branch
            par_regs = nc.alloc_registers("par_regs")
            nc.regs_load(par_regs, par_d.ap()[0:1, 0:1])
            par = nc.snap(par_regs, donate=True, min_val=0, max_val=1)

            # den accumulates (ones/128).T @ et so rbs = recip gives 128/den,
            # folding the fp8 weight scale (x128) into the reciprocal for free
            ones_sq = pp.tile([128, 128], b16, tag="ones_sq", bufs=1)
            nc.vector.memset(ones_sq[:], 1.0 / 128)
            ones8 = pp.tile([128, 2, 128], f8, tag="ones8", bufs=1)
            nc.vector.memset(ones8[:], 1.0)
            tri_t = pp.tile([128, 256], b16, tag="tri", bufs=1)
            ident_t = tri_t[:, 0:128]
            lneg_t = tri_t[:, 128:256]
            # warm the PE HAM clock-gate while the first input DMAs land
            wups = psum()
            for i in range(16):
                nc.tensor.matmul(wups[:, 0:128], ones_sq[:], ones_sq[:],
                                 start=(i == 0), stop=(i == 15))

            # ---- input loads (arrival-ordered for phase-A pipelining).
            # Descriptor generation serializes per issuing queue (~0.7us per
            # dma_start), so the early loads fan out across engine queues.
            def chunks(dram, c0, c1, width):
                return dram.ap()[:, c0 * width:c1 * width].rearrange(
                    "p (c n) -> p c n", n=width)

            wzq_a = pr.tile([128, ND, D], b16, tag="wzq", bufs=1)
            qx_a = pr.tile([128, ND, 1024], b16, tag="qx", bufs=1)
            # single sync queue = priority order at HBM; 4-chunk granules
            # beat the ~0.65us/descriptor issue rate that single-a granules
            # paid, so phase A never catches up with the arrivals
            qxv = qxT_d.ap().rearrange("p (c n) -> p c n", n=1024)
            for h in range(4):
                aa = slice(h * 2, h * 2 + 2)
                nc.sync.dma_start(qx_a[:, aa, 0:512], qxv[:, aa, 0:512])
                nc.sync.dma_start(wzq_a[:, aa], chunks(wzq_d, h * 2, h * 2 + 2, D))
            nc.sync.dma_start(qx_a[:, :, 512:1024], qxv[:, :, 512:1024])
            # xT feeds the scores pass; host interleaves it so each 512-col
            # chunk is flat-contiguous (full-bandwidth descriptors). Pass 1
            # runs descending from tc15, so load high chunks first.
            xt_a = pb.tile([128, 4, ND, 512], b16, tag="xt", bufs=1)
            for cc in (3, 2, 1, 0):
                nc.sync.dma_start(
                    xt_a[:, cc],
                    xT_d.ap()[:, cc * 4096:(cc + 1) * 4096]
                    .rearrange("p (a n) -> p a n", n=512))
            # x natural layout [t, d] in fp8 feeds the A@X DoubleRow pass
            xn_a = pb.tile([128, NT, D], f8, tag="xn", bufs=1)
            nc.sync.dma_start(xn_a[:], chunks(xn_d, 0, NT, D))
            nc.sync.dma_start(tri_t[:], tri_d.ap())
            # b_ff1 laid out [128, ND]: bias column fc serves f-chunk fc
            bf1_t = pp.tile([128, ND], f32, tag="bf1", bufs=1)
            nc.sync.dma_start(bf1_t[:], bf1_d.ap().rearrange("c p -> p c"))
            wzq_t = [wzq_a[:, d] for d in range(ND)]
            qx = [qx_a[:, d] for d in range(ND)]

            def xts(d, tcn):
                j = tcn % 4
                return xt_a[:, tcn // 4, d, j * 128:(j + 1) * 128]

            # ---- phase A: uT[d, s] = sum_a wzq[a,d] * qxT[a,s] ----
            # a-outer in two sb-halves (8 PSUM banks each, all m per half):
            # compute starts once wzq[a0]+qx[a0,sb0] land, and the sb0 ut
            # evictions (which gate the first scores blocks) overlap the
            # whole sb1 half.
            ut = [pb.tile([128, 1024], b16, tag=f"ut{m}", bufs=1,
                          name=f"ut{m}") for m in range(ND)]

            def phase_a(sb, ms, ups):
                for a in range(ND):
                    for m in ms:
                        nc.tensor.matmul(
                            ups[m][:],
                            wzq_t[a][:, m * 128:(m + 1) * 128],
                            qx[a][:, sb * 512:(sb + 1) * 512],
                            start=(a == 0), stop=(a == ND - 1))

            def evict_u(sb, ms, ups):
                # alternate vector / scalar so the eviction chain halves
                for m in ms:
                    dst = ut[m][:, sb * 512:(sb + 1) * 512]
                    if m % 2 == 0:
                        nc.vector.tensor_copy(dst, ups[m][:])
                    else:
                        nc.scalar.activation(dst, ups[m][:], AF.Copy)

            def phase_a_all():
                # m-groups of (6, 2): the mm PSUM tag has 6 banks (den holds
                # 2), and the 6-group's per-granule consumption (~1.28us)
                # stays under the DMA arrival rate so the start never stalls
                for sb in range(2):
                    for ms in (range(6), range(6, ND)):
                        ups = {m: psum() for m in ms}
                        phase_a(sb, ms, ups)
                        evict_u(sb, ms, ups)

            # phase-C weights prefetch into the same right pool (wzq/qx stay
            # live through the in-branch phase-A tail; fp8 weights fit all)
            wl_a = pr.tile([128, ND, D], b16, tag="wl", bufs=1)
            nc.sync.dma_start(wl_a[:], chunks(wvl_d, 0, ND, D))
            wf1_a = pr.tile([128, ND, D], f8, tag="wf1", bufs=1)
            nc.sync.dma_start(wf1_a[:], chunks(wff1_d, 0, ND, D))
            wf2_a = pr.tile([128, ND, D], f8, tag="wf2", bufs=1)
            nc.sync.dma_start(wf2_a[:], chunks(wff2_d, 0, ND, D))
            wvl_t = [wl_a[:, d] for d in range(ND)]

            attn = [pr.tile([128, 1024], b16, tag=f"at{d}", bufs=1,
                            name=f"at{d}") for d in range(ND)]

            def phase_b(parity):
                DR = mybir.MatmulPerfMode.DoubleRow
                # normalized fp8 weights in DoubleRow pair layout, one
                # 512-col buffer per half; groups write their 256-col slice.
                # Diagonal-region pad columns must be zero for paired reads.
                et8 = {}
                for h in range(2):
                    t8 = pb.tile([128, NT // 2, 2, 512], f8, tag=f"et8_{h}",
                                 bufs=1, name=f"et8_{parity}_{h}")
                    et8[h] = t8
                    k0 = HALF_STARTS[parity][h] // 256
                    nc.gpsimd.memset(t8[:, k0, 0, 128:512], 0)
                    nc.gpsimd.memset(t8[:, k0, 1, 256:512], 0)
                    nc.gpsimd.memset(t8[:, k0 + 1, 0, 384:512], 0)

                # pass 1 per group: scoresT -> exp, with the diag mask folded
                # into the scores PSUM as one extra matmul (ident.T @ Lneg
                # adds -30000 below the diagonal) and den accumulated inline
                # one block behind the scores matmuls.
                et = {}
                rbs = {}

                def pass1(g):
                    g0 = GRP_STARTS[parity][g]
                    slots = _gslots(parity, g)[::-1]   # widest first
                    den_ps = psum_den()
                    for i, tcn in enumerate(slots):
                        w = _width(parity, g, tcn)
                        diag = _is_diag(parity, g, tcn)
                        scp = psum()
                        for d in range(ND):
                            nc.tensor.matmul(
                                scp[:, 0:w],
                                xts(d, tcn),
                                ut[d][:, g * GW:g * GW + w],
                                start=(d == 0),
                                stop=(d == ND - 1 and not diag))
                        if diag:
                            nc.tensor.matmul(
                                scp[:, w - 128:w], ident_t, lneg_t,
                                start=False, stop=True)
                        if i > 0:
                            pt = slots[i - 1]
                            pw = _width(parity, g, pt)
                            nc.tensor.matmul(
                                den_ps[:, 0:pw], ones_sq[:], et[(g, pt)][:],
                                start=(i == 1), stop=False)
                        e = pb.tile([128, w], b16, tag=f"et{g}_{tcn}",
                                    bufs=1, name=f"et{parity}_{g}_{tcn}")
                        et[(g, tcn)] = e
                        nc.scalar.activation(e[:], scp[:, 0:w], AF.Exp)
                    lt = slots[-1]
                    lw = _width(parity, g, lt)
                    nc.tensor.matmul(
                        den_ps[:, 0:lw], ones_sq[:], et[(g, lt)][:],
                        start=(len(slots) == 1), stop=True)
                    r = sp.tile([128, GW], f32, tag="rbs", bufs=2,
                                name=f"rbs{parity}_{g}")
                    nc.vector.reciprocal_approx_fast(r[:], den_ps[:, 0:GW])
                    rbs[g] = r

                def quantize(g):
                    # et8 = et * (128/den) into the half buffer's 256-col
                    # slice, alternating vector/gpsimd
                    off = (g % 2) * 256
                    for i, tcn in enumerate(_gslots(parity, g)):
                        w = _width(parity, g, tcn)
                        eng = nc.vector if i % 2 == 0 else nc.gpsimd
                        eng.tensor_mul(
                            et8[g // 2][:, tcn // 2, tcn % 2, off:off + w],
                            et[(g, tcn)][:], rbs[g][:, 0:w])

                def pass2(h):
                    pairs = _hpairs(parity, h)[::-1]   # widest first

                    def pw(k):
                        return _hwidth(parity, h, 2 * k + 1)

                    den2 = psum_den()
                    for i, k in enumerate(pairs):
                        nc.tensor.matmul(
                            den2[:, 0:pw(k)], ones8[:],
                            et8[h][:, k, :, 0:pw(k)],
                            start=(i == 0), stop=(i == len(pairs) - 1),
                            perf_mode=DR)
                    r2 = sp.tile([128, 512], f32, tag="rbs2", bufs=2,
                                 name=f"rbs2{parity}_{h}")
                    nc.vector.reciprocal_approx_fast(r2[:], den2[:])
                    for dc in range(ND):
                        axp = psum()
                        for i, k in enumerate(pairs):
                            nc.tensor.matmul(
                                axp[:, 0:pw(k)],
                                xn_a[:, 2 * k:2 * k + 2,
                                     dc * 128:(dc + 1) * 128],
                                et8[h][:, k, :, 0:pw(k)],
                                start=(i == 0), stop=(i == len(pairs) - 1),
                                perf_mode=DR)
                        # PSUM reads are DVE-only (GpSimd can't touch PSUM)
                        nc.vector.tensor_mul(
                            attn[dc][:, h * 512:(h + 1) * 512],
                            axp[:], r2[:])

                # software pipeline: quantize(g) (DVE+Pool) hides under
                # later groups' pass-1 and the other half's A@X PE work;
                # recips are approx_fast
                for g in range(4):
                    pass1(g)
                    quantize(g)
                for h in HALF_ORDER[parity]:
                    pass2(h)

            # the entire phase A + B sits inside both branch bodies; the
            # branch is resolved right after warmup dispatch, overlapping
            # the startup DMA wait instead of stalling the PE mid-kernel
            with tc.If(par < 1) as cmp:
                phase_a_all()
                phase_b(0)
            with cmp.Else():
                phase_a_all()
                phase_b(1)

            # ---- free pB (ut/xt/xn/et); left pool for phase-C tiles ----
            esB.close()
            esC = es.enter_context(ExitStack())
            pc = esC.enter_context(tc.tile_pool(name="pC", bufs=1))

            x2f = [pc.tile([128, 1024], f32, tag=f"x2f{d}", bufs=1,
                           name=f"x2f{d}") for d in range(ND)]
            # Both FFN GEMMs run in fp8 DoubleRow. Scale chain: x2f carries
            # 32x (host scaled wvl/xq by 32); x2b = x2f/32 is true x2 in fp8;
            # w_ff1/w_ff2 are host-scaled by 32 into fp8's normal range; the
            # relu's scale=1/32 keeps ht exact; the final 32x output factor
            # is divided out on the host.
            x2b_a = pc.tile([128, ND, 1024], f8, tag="x2b", bufs=1)
            ht_a = pc.tile([128, ND, 1024], f8, tag="ht", bufs=1)

            # s2-halved pipeline: ff2(0)'s output adds (DVE) overlap ff1(1)'s
            # matmuls, so only ff2(1)'s tail is exposed past the last matmul
            def wvl_half(s2):
                cc = slice(s2 * 512, (s2 + 1) * 512)
                for oc in range(ND):
                    cps = psum()
                    for d in range(ND):
                        nc.tensor.matmul(
                            cps[:],
                            wvl_t[d][:, oc * 128:(oc + 1) * 128],
                            attn[d][:, cc],
                            start=(d == 0), stop=(d == ND - 1))
                    xqt = sp.tile([128, 512], f32, tag="xqt", bufs=4,
                                  name=f"xqt{oc}_{s2}")
                    nc.sync.dma_start(
                        xqt[:],
                        xq_d.ap()[oc * 128:(oc + 1) * 128, cc])
                    nc.vector.tensor_add(x2f[oc][:, cc], cps[:], xqt[:])
                    # scalar engine does the fp8 extraction; DVE stays free
                    # for the adds and any phase-B eviction spill-over
                    nc.scalar.activation(x2b_a[:, oc, cc], x2f[oc][:, cc],
                                         AF.Copy, scale=1.0 / 32)

            def ff1_half(s2):
                cc = slice(s2 * 512, (s2 + 1) * 512)
                for fc in range(ND):
                    cps = psum()
                    for d2 in range(0, ND, 2):
                        nc.tensor.matmul(
                            cps[:],
                            wf1_a[:, d2:d2 + 2, fc * 128:(fc + 1) * 128],
                            x2b_a[:, d2:d2 + 2, cc],
                            start=(d2 == 0), stop=(d2 == ND - 2),
                            perf_mode=mybir.MatmulPerfMode.DoubleRow)
                    nc.scalar.activation(ht_a[:, fc, cc], cps[:], AF.Relu,
                                         bias=bf1_t[:, fc:fc + 1],
                                         scale=1.0 / 32)

            def ff2_half(s2):
                cc = slice(s2 * 512, (s2 + 1) * 512)
                for oc in range(ND):
                    cps = psum()
                    for f2 in range(0, ND, 2):
                        nc.tensor.matmul(
                            cps[:],
                            wf2_a[:, f2:f2 + 2, oc * 128:(oc + 1) * 128],
                            ht_a[:, f2:f2 + 2, cc],
                            start=(f2 == 0), stop=(f2 == ND - 2),
                            perf_mode=mybir.MatmulPerfMode.DoubleRow)
                    # bf16 staging tile (bufs=8: no reuse wait within a
                    # half) and per-oc DMA on alternating queues: no
                    # serialized descriptor chain, half the exit bytes
                    ot = sp.tile([128, 512], b16, tag="ot", bufs=8,
                                 name=f"ot{oc}_{s2}")
                    nc.vector.tensor_add(ot[:], cps[:], x2f[oc][:, cc])
                    eng = nc.sync if oc % 2 == 0 else nc.gpsimd
                    eng.dma_start(
                        outT_d.ap()[oc * 128:(oc + 1) * 128, cc], ot[:])

            wvl_half(0)
            wvl_half(1)
            ff1_half(0)
            ff2_half(0)
            ff1_half(1)
            ff2_half(1)

    nc.compile()
    return nc


def _get_program():
    global _COMPILED
    if _COMPILED is None:
        _COMPILED = _build_program()
    return _COMPILED


def _p128(arr):
    """[c*128, C] -> [128, c*C]: the on-chip chunked layout, so device DMAs
    are contiguous per partition."""
    c = arr.shape[0] // 128
    return np.ascontiguousarray(
        arr.reshape(c, 128, -1).transpose(1, 0, 2).reshape(128, -1))


def _p128_xt(xT):
    """xT [D, S] -> [128, (cc, a, 512)]: 512-col chunks flat-contiguous so
    each chunk loads with full-bandwidth descriptors."""
    a = xT.reshape(ND, 128, 4, 512)            # [a, p, cc, n]
    return np.ascontiguousarray(
        a.transpose(1, 2, 0, 3).reshape(128, -1))


def kernel(x, wqkv, w_lin, b_lin, w_ff1, b_ff1, w_ff2, b_ff2):
    from concourse.bass_utils import run_bass_kernel_spmd

    x = np.asarray(x, np.float32)
    wqkv = np.asarray(wqkv, np.float32)
    Wq = wqkv[:, :D].astype(np.float64)
    Wk = wqkv[:, D:2 * D].astype(np.float64)
    Wv = wqkv[:, 2 * D:].astype(np.float64)

    F8 = ml_dtypes.float8_e4m3
    wzq = _p128(((Wq @ Wk.T) / 2.0).astype(BF16))   # [a, d] natural layout
    # phase C carries a 32x scale (wvl, xq) so x2b = x2f/32 is exact x2;
    # both FFN weights are scaled by 32 into fp8's normal range; the final
    # 32x on the output is divided out below
    wvl = _p128((Wv @ np.asarray(w_lin, np.float64) * 32.0).astype(BF16))
    wff1 = _p128((np.asarray(w_ff1, np.float32) * 32.0).astype(F8))
    wff2 = _p128((np.asarray(w_ff2, np.float32) * 32.0).astype(F8))
    # ident | Lneg: the anti-causal diag mask as a PE-side score bias
    lneg = np.where(np.arange(128)[:, None] < np.arange(128)[None, :],
                    np.float32(-30000.0), np.float32(0.0))
    tri = np.concatenate([np.eye(128, dtype=np.float32), lneg],
                         axis=1).astype(BF16)

    in_maps = []
    qcols_by_parity = {
        0: np.r_[0:512, 1536:2048],
        1: np.r_[512:1536],
    }
    b_lin = np.asarray(b_lin, np.float32)
    b_ff1 = np.asarray(b_ff1, np.float32)
    b_ff2 = np.asarray(b_ff2, np.float32)
    bf1 = np.ascontiguousarray(b_ff1.reshape(ND, 128))
    for c in range(NCORES):
        b, h = c // 2, c % 2
        xT32 = np.ascontiguousarray(x[b].T)               # [D, S] f32
        qcols = qcols_by_parity[h]
        qxT32 = np.ascontiguousarray(xT32[:, qcols])      # [D, 1024]
        in_maps.append({
            "xT": _p128_xt(xT32.astype(BF16)),
            "xn": _p128(x[b].astype(F8)),                 # [S, D] natural
            "qxT": _p128(qxT32.astype(BF16)),
            "xq": (qxT32 + b_lin[:, None]) * 32.0,        # b_lin folded in
            "wzq": wzq,
            "wvl": wvl,
            "wff1": wff1,
            "wff2": wff2,
            "tri": tri,
            "bf1": bf1,
            "par": np.full((1, 1), h, np.uint32),
        })

    global _LAST_IN_MAPS
    _LAST_IN_MAPS = in_maps
    nc = _get_program()
    res = run_bass_kernel_spmd(nc, in_maps, core_ids=list(range(NCORES)))

    out = np.empty((B, S, D), np.float32)
    for c in range(NCORES):
        b, h = c // 2, c % 2
        ol = np.asarray(res.results[c]["outT"],
                        np.float32).T / 32.0              # [1024 s, D]
        if h == 0:
            out[b, 0:512] = ol[:512]
            out[b, 1536:2048] = ol[512:]
        else:
            out[b, 512:1536] = ol
    out += b_ff2[None, None, :]
    return out



# revision 33
# speedup vs baseline: 1.1349x; 1.0198x over previous
"""Trainium2 Bass kernel for nn_MultiHeadAttention_48395691492077.

Reference (B=4, S=2048, D=1024, single head, anti-causal triu mask):
    qkv = x @ wqkv; q,k,v = split(qkv)
    scores = triu(q @ k^T / sqrt(B));  masked softmax over keys t >= s
    x2  = softmax(scores) @ v @ w_lin + b_lin + x
    out = relu(x2 @ w_ff1 + b_ff1) @ w_ff2 + b_ff2 + x2

Sharding: 8 cores = 4 batches x 2 query-halves. Each core computes
attention + MLP for its own 1024 queries against the full 2048-key
sequence of its batch. The program is identical on all cores (SPMD);
per-core differences (which queries, which mask pattern) are carried in
the input data plus one branch on the query-parity register.

Device algebra (transposed; no on-chip transposes, no K/V projections):
    uT = wzq^T.T @ qxT            with wzq = (Wq @ Wk^T)/2  (host-fused)
    scoresT[t,s] = sum_d xT[d,t] * uT[d,s]     (keys are raw x!)
    expT = exp(scoresT); diagonal 128-strips *= lower-tri mask
    den[s]/128 broadcast = (ones/128).T @ expT (PE, inlined into pass 1)
    et8 = fp8(expT * 128/den)   (row-stochastic weights x128, fp8 range-safe)
    den2[s] = ones8.T @ et8     (fp8 DoubleRow; renormalizes the fp8
          quantization so attention stays exactly row-stochastic)
    H^T[d,s] = xn8[t,d].T @ et8 (A@X in fp8 DoubleRow; V proj deferred)
    attnT = H^T / den2
    x2T = wvl.T @ attnT + (xT + b_lin)  with wvl = Wv @ w_lin (host-fused:
          A@(X@Wv)@w_lin == (A@X)@(Wv@w_lin) by associativity)
    hT = relu(w_ff1.T @ x2T + b_ff1)
    outT = w_ff2.T @ hT + x2T               (+ b_ff2 added on host)
Blocks crossing the anti-causal diagonal use exact matmul widths
(128/256/384) instead of full 512; PSUM accumulation runs widest-first
so every column is initialized by the start=True matmul. A@X pairs
adjacent t-chunks for DoubleRow; diagonal blocks' tail columns are
zero-padded once so paired reads stay correct.
Matmul inputs are bf16 (fp32 PSUM accumulation) except A@X/FFN (fp8
DoubleRow); residuals are fp32. ff2 adds in place into x2f and streams
per-oc output DMAs on alternating queues to keep the tail short.
"""

import numpy as np
import ml_dtypes

B, S, D = 4, 2048, 1024
NCORES = 8
BF16 = ml_dtypes.bfloat16

NT = S // 128            # 16 t-chunks
ND = D // 128            # 8 chunks of 128 along any D-sized dim

# global query-column starts of the four 256-query groups per parity.
# Together they tile the same query sets as the old 512-col halves, so the
# host-side qcols mapping is unchanged. 256-col groups give the phase-B
# pipeline four units: quantize(g) hides under pass1(g+1)/A@X(g-1) PE work.
GRP_STARTS = {0: (0, 256, 1536, 1792), 1: (512, 768, 1024, 1280)}
GW = 256                 # group width (queries per group)
# 512-query halves (pairs of groups) used by den2/A@X so the DoubleRow
# matmuls run at FD 512 (FD 256 is LDWEIGHTS-bound: 182ns vs 110ns/pair).
# Order per parity: the half whose quantize finishes first goes first.
HALF_STARTS = {0: (0, 1536), 1: (512, 1024)}


def _gslots(parity, g):
    return list(range(GRP_STARTS[parity][g] // 128, NT))


def _width(parity, g, tc):
    """Valid column count of block (g, tc): cols [0, w) of the 256."""
    return min(GW, 128 * tc - GRP_STARTS[parity][g] + 128)


def _is_diag(parity, g, tc):
    """Block whose last 128 columns lie on the anti-causal diagonal."""
    return tc - GRP_STARTS[parity][g] // 128 < 2


def _hpairs(parity, h):
    return list(range(HALF_STARTS[parity][h] // 256, NT // 2))


def _hwidth(parity, h, tc):
    return min(512, 128 * tc - HALF_STARTS[parity][h] + 128)


_COMPILED = None
_LAST_IN_MAPS = None


def _build_program():
    from contextlib import ExitStack
    import concourse.bacc as bacc
    import concourse.mybir as mybir
    import concourse.tile as tile

    f32 = mybir.dt.float32
    b16 = mybir.dt.bfloat16
    AF = mybir.ActivationFunctionType

    nc = bacc.Bacc("TRN2", target_bir_lowering=False, debug=False,
                   num_devices=NCORES)

    # all big inputs arrive pre-arranged on the host into the on-chip
    # [128, chunk, free] layout so every DMA is contiguous per partition
    f8 = mybir.dt.float8e4
    xT_d = nc.dram_tensor("xT", [128, ND * S], b16, kind="ExternalInput")
    xn_d = nc.dram_tensor("xn", [128, NT * D], f8, kind="ExternalInput")
    qxT_d = nc.dram_tensor("qxT", [128, ND * 1024], b16, kind="ExternalInput")
    xq_d = nc.dram_tensor("xq", [D, 1024], f32, kind="ExternalInput")
    wzq_d = nc.dram_tensor("wzq", [128, ND * D], b16, kind="ExternalInput")
    wvl_d = nc.dram_tensor("wvl", [128, ND * D], b16, kind="ExternalInput")
    wff1_d = nc.dram_tensor("wff1", [128, ND * D], f8, kind="ExternalInput")
    wff2_d = nc.dram_tensor("wff2", [128, ND * D], f8, kind="ExternalInput")
    # ident | Lneg: identity and the strictly-lower -30000 mask-bias, both
    # bf16; the diag mask is applied as one extra PE matmul into the scores
    # PSUM (ident.T @ Lneg adds -30000 below the diagonal) instead of a
    # post-exp vector multiply
    tri_d = nc.dram_tensor("tri", [128, 256], b16, kind="ExternalInput")
    par_d = nc.dram_tensor("par", [1, 1], mybir.dt.uint32, kind="ExternalInput")
    bf1_d = nc.dram_tensor("bf1", [ND, 128], f32, kind="ExternalInput")
    # bf16 output halves the exit DMA; host upcasts (costs ~2e-4 rel err)
    outT_d = nc.dram_tensor("outT", [D, 1024], b16, kind="ExternalOutput")

    with tile.TileContext(nc) as tc:
        es = ExitStack()
        with es:
            pp = es.enter_context(tc.tile_pool(name="persist", bufs=1))
            sp = es.enter_context(tc.tile_pool(name="stream", bufs=2))
            ps = es.enter_context(
                tc.tile_pool(name="ps", bufs=8, space="PSUM"))
            esB = es.enter_context(ExitStack())
            pb = esB.enter_context(tc.tile_pool(name="pB", bufs=1))
            pr = es.enter_context(tc.tile_pool(name="pAC", bufs=1,
                                               side="right"))

            def psum():
                # single 8-bank ring; den/den2 allocate from it too, at
                # program points where their slot isn't re-requested until
                # >=8 allocations later (they're freed by then)
                t = ps.tile([128, 512], f32, tag="mm", bufs=8, name="mmps")
                return t

            # ---- constants ----
            # load the parity register up front so every engine sequencer
            # resolves it during the startup DMA wait, not at the branch
            par_regs = nc.alloc_registers("par_regs")
            nc.regs_load(par_regs, par_d.ap()[0:1, 0:1])
            par = nc.snap(par_regs, donate=True, min_val=0, max_val=1)

            # den accumulates (ones/128).T @ et so rbs = recip gives 128/den,
            # folding the fp8 weight scale (x128) into the reciprocal for free
            ones_sq = pp.tile([128, 128], b16, tag="ones_sq", bufs=1)
            nc.vector.memset(ones_sq[:], 1.0 / 128)
            ones8 = pp.tile([128, 2, 128], f8, tag="ones8", bufs=1)
            nc.vector.memset(ones8[:], 1.0)
            tri_t = pp.tile([128, 256], b16, tag="tri", bufs=1)
            ident_t = tri_t[:, 0:128]
            lneg_t = tri_t[:, 128:256]
            # warm the PE HAM clock-gate while the first input DMAs land
            wups = psum()
            for i in range(16):
                nc.tensor.matmul(wups[:, 0:128], ones_sq[:], ones_sq[:],
                                 start=(i == 0), stop=(i == 15))

            # ---- input loads (arrival-ordered for phase-A pipelining).
            # Descriptor generation serializes per issuing queue (~0.7us per
            # dma_start), so the early loads fan out across engine queues.
            def chunks(dram, c0, c1, width):
                return dram.ap()[:, c0 * width:c1 * width].rearrange(
                    "p (c n) -> p c n", n=width)

            wzq_a = pr.tile([128, ND, D], b16, tag="wzq", bufs=1)
            qx_a = pr.tile([128, ND, 1024], b16, tag="qx", bufs=1)
            # single sync queue = priority order at HBM; 4-chunk granules
            # beat the ~0.65us/descriptor issue rate that single-a granules
            # paid, so phase A never catches up with the arrivals
            # single-a granules: issue rate (~0.65us each, alternating) stays
            # ahead of the 8-matmul-per-a consumption rate (~1.7us)
            qxv = qxT_d.ap().rearrange("p (c n) -> p c n", n=1024)
            for a in range(ND):
                nc.sync.dma_start(wzq_a[:, a:a + 1],
                                  chunks(wzq_d, a, a + 1, D))
                nc.sync.dma_start(qx_a[:, a:a + 1, 0:512],
                                  qxv[:, a:a + 1, 0:512])
            nc.sync.dma_start(qx_a[:, :, 512:1024], qxv[:, :, 512:1024])
            # xT feeds the scores pass; host interleaves it so each 512-col
            # chunk is flat-contiguous (full-bandwidth descriptors). Pass 1
            # runs descending from tc15, so load high chunks first.
            xt_a = pb.tile([128, 4, ND, 512], b16, tag="xt", bufs=1)
            for cc in (3, 2, 1, 0):
                nc.sync.dma_start(
                    xt_a[:, cc],
                    xT_d.ap()[:, cc * 4096:(cc + 1) * 4096]
                    .rearrange("p (a n) -> p a n", n=512))
            # x natural layout [t, d] in fp8 feeds the A@X DoubleRow pass
            xn_a = pb.tile([128, NT, D], f8, tag="xn", bufs=1)
            nc.sync.dma_start(xn_a[:], chunks(xn_d, 0, NT, D))
            nc.sync.dma_start(tri_t[:], tri_d.ap())
            # b_ff1 laid out [128, ND]: bias column fc serves f-chunk fc
            bf1_t = pp.tile([128, ND], f32, tag="bf1", bufs=1)
            nc.sync.dma_start(bf1_t[:], bf1_d.ap().rearrange("c p -> p c"))
            wzq_t = [wzq_a[:, d] for d in range(ND)]
            qx = [qx_a[:, d] for d in range(ND)]

            def xts(d, tcn):
                j = tcn % 4
                return xt_a[:, tcn // 4, d, j * 128:(j + 1) * 128]

            # ---- phase A: uT[d, s] = sum_a wzq[a,d] * qxT[a,s] ----
            # a-outer in two sb-halves (8 PSUM banks each, all m per half):
            # compute starts once wzq[a0]+qx[a0,sb0] land, and the sb0 ut
            # evictions (which gate the first scores blocks) overlap the
            # whole sb1 half.
            ut = [pb.tile([128, 1024], b16, tag=f"ut{m}", bufs=1,
                          name=f"ut{m}") for m in range(ND)]

            def phase_a(sb, ms, ups):
                for a in range(ND):
                    for m in ms:
                        nc.tensor.matmul(
                            ups[m][:],
                            wzq_t[a][:, m * 128:(m + 1) * 128],
                            qx[a][:, sb * 512:(sb + 1) * 512],
                            start=(a == 0), stop=(a == ND - 1))

            def evict_u(sb, ms, ups):
                # alternate vector / scalar so the eviction chain halves
                for m in ms:
                    dst = ut[m][:, sb * 512:(sb + 1) * 512]
                    if m % 2 == 0:
                        nc.vector.tensor_copy(dst, ups[m][:])
                    else:
                        nc.scalar.activation(dst, ups[m][:], AF.Copy)

            def phase_a_all():
                for sb in range(2):
                    ms = range(ND)
                    ups = {m: psum() for m in ms}
                    phase_a(sb, ms, ups)
                    evict_u(sb, ms, ups)

            # phase-C weights prefetch into the same right pool (wzq/qx stay
            # live through the in-branch phase-A tail; fp8 weights fit all)
            wl_a = pr.tile([128, ND, D], b16, tag="wl", bufs=1)
            nc.sync.dma_start(wl_a[:], chunks(wvl_d, 0, ND, D))
            wf1_a = pr.tile([128, ND, D], f8, tag="wf1", bufs=1)
            nc.sync.dma_start(wf1_a[:], chunks(wff1_d, 0, ND, D))
            wf2_a = pr.tile([128, ND, D], f8, tag="wf2", bufs=1)
            nc.sync.dma_start(wf2_a[:], chunks(wff2_d, 0, ND, D))
            wvl_t = [wl_a[:, d] for d in range(ND)]

            attn = [pr.tile([128, 1024], b16, tag=f"at{d}", bufs=1,
                            name=f"at{d}") for d in range(ND)]

            def phase_b(parity):
                DR = mybir.MatmulPerfMode.DoubleRow
                # normalized fp8 weights in DoubleRow pair layout, one
                # 512-col buffer per half; groups write their 256-col slice.
                # Diagonal-region pad columns must be zero for paired reads.
                et8 = {}
                for h in range(2):
                    t8 = pb.tile([128, NT // 2, 2, 512], f8, tag=f"et8_{h}",
                                 bufs=1, name=f"et8_{parity}_{h}")
                    et8[h] = t8
                    k0 = HALF_STARTS[parity][h] // 256
                    nc.gpsimd.memset(t8[:, k0, 0, 128:512], 0)
                    nc.gpsimd.memset(t8[:, k0, 1, 256:512], 0)
                    nc.gpsimd.memset(t8[:, k0 + 1, 0, 384:512], 0)

                # pass 1 per group: scoresT -> exp, with the diag mask folded
                # into the scores PSUM as one extra matmul (ident.T @ Lneg
                # adds -30000 below the diagonal) and den accumulated inline
                # one block behind the scores matmuls.
                et = {}
                rbs = {}

                def pass1(g):
                    slots = _gslots(parity, g)[::-1]   # widest first
                    for tcn in slots:
                        w = _width(parity, g, tcn)
                        diag = _is_diag(parity, g, tcn)
                        scp = psum()
                        for d in range(ND):
                            nc.tensor.matmul(
                                scp[:, 0:w],
                                xts(d, tcn),
                                ut[d][:, g * GW:g * GW + w],
                                start=(d == 0),
                                stop=(d == ND - 1 and not diag))
                        if diag:
                            nc.tensor.matmul(
                                scp[:, w - 128:w], ident_t, lneg_t,
                                start=False, stop=True)
                        e = pb.tile([128, w], b16, tag=f"et{g}_{tcn}",
                                    bufs=1, name=f"et{parity}_{g}_{tcn}")
                        et[(g, tcn)] = e
                        nc.scalar.activation(e[:], scp[:, 0:w], AF.Exp)
                    # den as an end-of-group chain: the mm-ring slot it
                    # takes isn't requested again until deep into the next
                    # group's blocks, by which time the recip has read it
                    den_ps = psum()
                    for i, tcn in enumerate(slots):
                        pw = _width(parity, g, tcn)
                        nc.tensor.matmul(
                            den_ps[:, 0:pw], ones_sq[:], et[(g, tcn)][:],
                            start=(i == 0), stop=(i == len(slots) - 1))
                    r = sp.tile([128, GW], f32, tag="rbs", bufs=2,
                                name=f"rbs{parity}_{g}")
                    nc.vector.reciprocal_approx_fast(r[:], den_ps[:, 0:GW])
                    rbs[g] = r

                def quantize(g):
                    # et8 = et * (128/den) into the half buffer's 256-col
                    # slice, alternating vector/gpsimd
                    off = (g % 2) * 256
                    for i, tcn in enumerate(_gslots(parity, g)):
                        w = _width(parity, g, tcn)
                        eng = nc.vector if i % 2 == 0 else nc.gpsimd
                        eng.tensor_mul(
                            et8[g // 2][:, tcn // 2, tcn % 2, off:off + w],
                            et[(g, tcn)][:], rbs[g][:, 0:w])

                def pass2(h):
                    pairs = _hpairs(parity, h)[::-1]   # widest first

                    def pw(k):
                        return _hwidth(parity, h, 2 * k + 1)

                    den2 = psum()
                    for i, k in enumerate(pairs):
                        nc.tensor.matmul(
                            den2[:, 0:pw(k)], ones8[:],
                            et8[h][:, k, :, 0:pw(k)],
                            start=(i == 0), stop=(i == len(pairs) - 1),
                            perf_mode=DR)
                    r2 = sp.tile([128, 512], f32, tag="rbs2", bufs=2,
                                 name=f"rbs2{parity}_{h}")
                    nc.vector.reciprocal_approx_fast(r2[:], den2[:])
                    for dc in range(ND):
                        axp = psum()
                        for i, k in enumerate(pairs):
                            nc.tensor.matmul(
                                axp[:, 0:pw(k)],
                                xn_a[:, 2 * k:2 * k + 2,
                                     dc * 128:(dc + 1) * 128],
                                et8[h][:, k, :, 0:pw(k)],
                                start=(i == 0), stop=(i == len(pairs) - 1),
                                perf_mode=DR)
                        # PSUM reads are DVE-only (GpSimd can't touch PSUM)
                        nc.vector.tensor_mul(
                            attn[dc][:, h * 512:(h + 1) * 512],
                            axp[:], r2[:])

                # software pipeline: quantize(g) (DVE+Pool) hides under
                # later groups' pass-1 and the other half's A@X PE work;
                # recips are approx_fast
                for g in range(4):
                    pass1(g)
                    quantize(g)
                for h in (0, 1):
                    pass2(h)

            # the entire phase A + B sits inside both branch bodies; the
            # branch is resolved right after warmup dispatch, overlapping
            # the startup DMA wait instead of stalling the PE mid-kernel
            with tc.If(par < 1) as cmp:
                phase_a_all()
                phase_b(0)
            with cmp.Else():
                phase_a_all()
                phase_b(1)

            # ---- free pB (ut/xt/xn/et); left pool for phase-C tiles ----
            esB.close()
            esC = es.enter_context(ExitStack())
            pc = esC.enter_context(tc.tile_pool(name="pC", bufs=1))

            x2f = [pc.tile([128, 1024], f32, tag=f"x2f{d}", bufs=1,
                           name=f"x2f{d}") for d in range(ND)]
            # Both FFN GEMMs run in fp8 DoubleRow. Scale chain: x2f carries
            # 32x (host scaled wvl/xq by 32); x2b = x2f/32 is true x2 in fp8;
            # w_ff1/w_ff2 are host-scaled by 32 into fp8's normal range; the
            # relu's scale=1/32 keeps ht exact; the final 32x output factor
            # is divided out on the host.
            x2b_a = pc.tile([128, ND, 1024], f8, tag="x2b", bufs=1)
            ht_a = pc.tile([128, ND, 1024], f8, tag="ht", bufs=1)

            # s2-halved pipeline: ff2(0)'s output adds (DVE) overlap ff1(1)'s
            # matmuls, so only ff2(1)'s tail is exposed past the last matmul
            def wvl_half(s2):
                cc = slice(s2 * 512, (s2 + 1) * 512)
                for oc in range(ND):
                    cps = psum()
                    for d in range(ND):
                        nc.tensor.matmul(
                            cps[:],
                            wvl_t[d][:, oc * 128:(oc + 1) * 128],
                            attn[d][:, cc],
                            start=(d == 0), stop=(d == ND - 1))
                    xqt = sp.tile([128, 512], f32, tag="xqt", bufs=4,
                                  name=f"xqt{oc}_{s2}")
                    nc.sync.dma_start(
                        xqt[:],
                        xq_d.ap()[oc * 128:(oc + 1) * 128, cc])
                    nc.vector.tensor_add(x2f[oc][:, cc], cps[:], xqt[:])
                    # scalar engine does the fp8 extraction; DVE stays free
                    # for the adds and any phase-B eviction spill-over
                    nc.scalar.activation(x2b_a[:, oc, cc], x2f[oc][:, cc],
                                         AF.Copy, scale=1.0 / 32)

            def ff1_half(s2):
                cc = slice(s2 * 512, (s2 + 1) * 512)
                for fc in range(ND):
                    cps = psum()
                    for d2 in range(0, ND, 2):
                        nc.tensor.matmul(
                            cps[:],
                            wf1_a[:, d2:d2 + 2, fc * 128:(fc + 1) * 128],
                            x2b_a[:, d2:d2 + 2, cc],
                            start=(d2 == 0), stop=(d2 == ND - 2),
                            perf_mode=mybir.MatmulPerfMode.DoubleRow)
                    nc.scalar.activation(ht_a[:, fc, cc], cps[:], AF.Relu,
                                         bias=bf1_t[:, fc:fc + 1],
                                         scale=1.0 / 32)

            def ff2_half(s2):
                cc = slice(s2 * 512, (s2 + 1) * 512)
                for oc in range(ND):
                    cps = psum()
                    for f2 in range(0, ND, 2):
                        nc.tensor.matmul(
                            cps[:],
                            wf2_a[:, f2:f2 + 2, oc * 128:(oc + 1) * 128],
                            ht_a[:, f2:f2 + 2, cc],
                            start=(f2 == 0), stop=(f2 == ND - 2),
                            perf_mode=mybir.MatmulPerfMode.DoubleRow)
                    # bf16 staging tile (bufs=8: no reuse wait within a
                    # half) and per-oc DMA on alternating queues: no
                    # serialized descriptor chain, half the exit bytes
                    ot = sp.tile([128, 512], b16, tag="ot", bufs=8,
                                 name=f"ot{oc}_{s2}")
                    nc.vector.tensor_add(ot[:], cps[:], x2f[oc][:, cc])
                    if s2 == 0:
                        eng = nc.sync if oc % 2 == 0 else nc.gpsimd
                    else:
                        # scalar is free in the last half: a 3rd queue
                        # drains the exit transfers sooner
                        eng = (nc.sync, nc.gpsimd, nc.scalar)[oc % 3]
                    eng.dma_start(
                        outT_d.ap()[oc * 128:(oc + 1) * 128, cc], ot[:])

            wvl_half(0)
            wvl_half(1)
            ff1_half(0)
            ff2_half(0)
            ff1_half(1)
            ff2_half(1)

    nc.compile()
    return nc


def _get_program():
    global _COMPILED
    if _COMPILED is None:
        _COMPILED = _build_program()
    return _COMPILED


def _p128(arr):
    """[c*128, C] -> [128, c*C]: the on-chip chunked layout, so device DMAs
    are contiguous per partition."""
    c = arr.shape[0] // 128
    return np.ascontiguousarray(
        arr.reshape(c, 128, -1).transpose(1, 0, 2).reshape(128, -1))


def _p128_xt(xT):
    """xT [D, S] -> [128, (cc, a, 512)]: 512-col chunks flat-contiguous so
    each chunk loads with full-bandwidth descriptors."""
    a = xT.reshape(ND, 128, 4, 512)            # [a, p, cc, n]
    return np.ascontiguousarray(
        a.transpose(1, 2, 0, 3).reshape(128, -1))


def kernel(x, wqkv, w_lin, b_lin, w_ff1, b_ff1, w_ff2, b_ff2):
    from concourse.bass_utils import run_bass_kernel_spmd

    x = np.asarray(x, np.float32)
    wqkv = np.asarray(wqkv, np.float32)
    Wq = wqkv[:, :D].astype(np.float64)
    Wk = wqkv[:, D:2 * D].astype(np.float64)
    Wv = wqkv[:, 2 * D:].astype(np.float64)

    F8 = ml_dtypes.float8_e4m3
    wzq = _p128(((Wq @ Wk.T) / 2.0).astype(BF16))   # [a, d] natural layout
    # phase C carries a 32x scale (wvl, xq) so x2b = x2f/32 is exact x2;
    # both FFN weights are scaled by 32 into fp8's normal range; the final
    # 32x on the output is divided out below
    wvl = _p128((Wv @ np.asarray(w_lin, np.float64) * 32.0).astype(BF16))
    wff1 = _p128((np.asarray(w_ff1, np.float32) * 32.0).astype(F8))
    wff2 = _p128((np.asarray(w_ff2, np.float32) * 32.0).astype(F8))
    # ident | Lneg: the anti-causal diag mask as a PE-side score bias
    lneg = np.where(np.arange(128)[:, None] < np.arange(128)[None, :],
                    np.float32(-30000.0), np.float32(0.0))
    tri = np.concatenate([np.eye(128, dtype=np.float32), lneg],
                         axis=1).astype(BF16)

    in_maps = []
    qcols_by_parity = {
        0: np.r_[0:512, 1536:2048],
        1: np.r_[512:1536],
    }
    b_lin = np.asarray(b_lin, np.float32)
    b_ff1 = np.asarray(b_ff1, np.float32)
    b_ff2 = np.asarray(b_ff2, np.float32)
    bf1 = np.ascontiguousarray(b_ff1.reshape(ND, 128))
    for c in range(NCORES):
        b, h = c // 2, c % 2
        xT32 = np.ascontiguousarray(x[b].T)               # [D, S] f32
        qcols = qcols_by_parity[h]
        qxT32 = np.ascontiguousarray(xT32[:, qcols])      # [D, 1024]
        in_maps.append({
            "xT": _p128_xt(xT32.astype(BF16)),
            "xn": _p128(x[b].astype(F8)),                 # [S, D] natural
            "qxT": _p128(qxT32.astype(BF16)),
            "xq": (qxT32 + b_lin[:, None]) * 32.0,        # b_lin folded in
            "wzq": wzq,
            "wvl": wvl,
            "wff1": wff1,
            "wff2": wff2,
            "tri": tri,
            "bf1": bf1,
            "par": np.full((1, 1), h, np.uint32),
        })

    global _LAST_IN_MAPS
    _LAST_IN_MAPS = in_maps
    nc = _get_program()
    res = run_bass_kernel_spmd(nc, in_maps, core_ids=list(range(NCORES)))

    out = np.empty((B, S, D), np.float32)
    for c in range(NCORES):
        b, h = c // 2, c % 2
        ol = np.asarray(res.results[c]["outT"],
                        np.float32).T / 32.0              # [1024 s, D]
        if h == 0:
            out[b, 0:512] = ol[:512]
            out[b, 1536:2048] = ol[512:]
        else:
            out[b, 512:1536] = ol
    out += b_ff2[None, None, :]
    return out



# revision 36
# speedup vs baseline: 1.1427x; 1.0069x over previous
"""Trainium2 Bass kernel for nn_MultiHeadAttention_48395691492077.

Reference (B=4, S=2048, D=1024, single head, anti-causal triu mask):
    qkv = x @ wqkv; q,k,v = split(qkv)
    scores = triu(q @ k^T / sqrt(B));  masked softmax over keys t >= s
    x2  = softmax(scores) @ v @ w_lin + b_lin + x
    out = relu(x2 @ w_ff1 + b_ff1) @ w_ff2 + b_ff2 + x2

Sharding: 8 cores = 4 batches x 2 query-halves. Each core computes
attention + MLP for its own 1024 queries against the full 2048-key
sequence of its batch. The program is identical on all cores (SPMD);
per-core differences (which queries, which mask pattern) are carried in
the input data plus one branch on the query-parity register.

Device algebra (transposed; no on-chip transposes, no K/V projections):
    uT = wzq^T.T @ qxT            with wzq = (Wq @ Wk^T)/2  (host-fused)
    scoresT[t,s] = sum_d xT[d,t] * uT[d,s]     (keys are raw x!)
    expT = exp(scoresT); diagonal 128-strips *= lower-tri mask
    den[s]/128 broadcast = (ones/128).T @ expT (PE, inlined into pass 1)
    et8 = fp8(expT * 128/den)   (row-stochastic weights x128, fp8 range-safe)
    den2[s] = ones8.T @ et8     (fp8 DoubleRow; renormalizes the fp8
          quantization so attention stays exactly row-stochastic)
    H^T[d,s] = xn8[t,d].T @ et8 (A@X in fp8 DoubleRow; V proj deferred)
    attnT = H^T / den2
    x2T = wvl.T @ attnT + (xT + b_lin)  with wvl = Wv @ w_lin (host-fused:
          A@(X@Wv)@w_lin == (A@X)@(Wv@w_lin) by associativity)
    hT = relu(w_ff1.T @ x2T + b_ff1)
    outT = w_ff2.T @ hT + x2T               (+ b_ff2 added on host)
Blocks crossing the anti-causal diagonal use exact matmul widths
(128/256/384) instead of full 512; PSUM accumulation runs widest-first
so every column is initialized by the start=True matmul. A@X pairs
adjacent t-chunks for DoubleRow; diagonal blocks' tail columns are
zero-padded once so paired reads stay correct.
Matmul inputs are bf16 (fp32 PSUM accumulation) except A@X/FFN (fp8
DoubleRow); residuals are fp32. ff2 adds in place into x2f and streams
per-oc output DMAs on alternating queues to keep the tail short.
"""

import numpy as np
import ml_dtypes

B, S, D = 4, 2048, 1024
NCORES = 8
BF16 = ml_dtypes.bfloat16

NT = S // 128            # 16 t-chunks
ND = D // 128            # 8 chunks of 128 along any D-sized dim

# global query-column starts of the four 256-query groups per parity.
# Together they tile the same query sets as the old 512-col halves, so the
# host-side qcols mapping is unchanged. 256-col groups give the phase-B
# pipeline four units: quantize(g) hides under pass1(g+1)/A@X(g-1) PE work.
GRP_STARTS = {0: (0, 256, 1536, 1792), 1: (512, 768, 1024, 1280)}
GW = 256                 # group width (queries per group)
# 512-query halves (pairs of groups) used by den2/A@X so the DoubleRow
# matmuls run at FD 512 (FD 256 is LDWEIGHTS-bound: 182ns vs 110ns/pair).
# Order per parity: the half whose quantize finishes first goes first.
HALF_STARTS = {0: (0, 1536), 1: (512, 1024)}


def _gslots(parity, g):
    return list(range(GRP_STARTS[parity][g] // 128, NT))


def _width(parity, g, tc):
    """Valid column count of block (g, tc): cols [0, w) of the 256."""
    return min(GW, 128 * tc - GRP_STARTS[parity][g] + 128)


def _is_diag(parity, g, tc):
    """Block whose last 128 columns lie on the anti-causal diagonal."""
    return tc - GRP_STARTS[parity][g] // 128 < 2


def _hpairs(parity, h):
    return list(range(HALF_STARTS[parity][h] // 256, NT // 2))


def _hwidth(parity, h, tc):
    return min(512, 128 * tc - HALF_STARTS[parity][h] + 128)


_COMPILED = None
_LAST_IN_MAPS = None


def _build_program():
    from contextlib import ExitStack
    import concourse.bacc as bacc
    import concourse.mybir as mybir
    import concourse.tile as tile

    f32 = mybir.dt.float32
    b16 = mybir.dt.bfloat16
    AF = mybir.ActivationFunctionType

    nc = bacc.Bacc("TRN2", target_bir_lowering=False, debug=False,
                   num_devices=NCORES)

    # all big inputs arrive pre-arranged on the host into the on-chip
    # [128, chunk, free] layout so every DMA is contiguous per partition
    f8 = mybir.dt.float8e4
    xT_d = nc.dram_tensor("xT", [128, ND * S], b16, kind="ExternalInput")
    xn_d = nc.dram_tensor("xn", [128, NT * D], f8, kind="ExternalInput")
    qxT_d = nc.dram_tensor("qxT", [128, ND * 1024], b16, kind="ExternalInput")
    xq_d = nc.dram_tensor("xq", [D, 1024], f32, kind="ExternalInput")
    wzq_d = nc.dram_tensor("wzq", [128, ND * D], b16, kind="ExternalInput")
    wvl_d = nc.dram_tensor("wvl", [128, ND * D], b16, kind="ExternalInput")
    wff1_d = nc.dram_tensor("wff1", [128, ND * D], f8, kind="ExternalInput")
    wff2_d = nc.dram_tensor("wff2", [128, ND * D], f8, kind="ExternalInput")
    # ident | Lneg: identity and the strictly-lower -30000 mask-bias, both
    # bf16; the diag mask is applied as one extra PE matmul into the scores
    # PSUM (ident.T @ Lneg adds -30000 below the diagonal) instead of a
    # post-exp vector multiply
    tri_d = nc.dram_tensor("tri", [128, 256], b16, kind="ExternalInput")
    par_d = nc.dram_tensor("par", [1, 1], mybir.dt.uint32, kind="ExternalInput")
    bf1_d = nc.dram_tensor("bf1", [ND, 128], f32, kind="ExternalInput")
    # bf16 output halves the exit DMA; host upcasts (costs ~2e-4 rel err)
    outT_d = nc.dram_tensor("outT", [D, 1024], b16, kind="ExternalOutput")

    with tile.TileContext(nc) as tc:
        es = ExitStack()
        with es:
            pp = es.enter_context(tc.tile_pool(name="persist", bufs=1))
            sp = es.enter_context(tc.tile_pool(name="stream", bufs=2))
            ps = es.enter_context(
                tc.tile_pool(name="ps", bufs=8, space="PSUM"))
            esB = es.enter_context(ExitStack())
            pb = esB.enter_context(tc.tile_pool(name="pB", bufs=1))
            pr = es.enter_context(tc.tile_pool(name="pAC", bufs=1,
                                               side="right"))

            def psum():
                # single 8-bank ring; den/den2 allocate from it too, at
                # program points where their slot isn't re-requested until
                # >=8 allocations later (they're freed by then)
                t = ps.tile([128, 512], f32, tag="mm", bufs=8, name="mmps")
                return t

            # ---- constants ----
            # load the parity register up front so every engine sequencer
            # resolves it during the startup DMA wait, not at the branch
            par_regs = nc.alloc_registers("par_regs")
            nc.regs_load(par_regs, par_d.ap()[0:1, 0:1])
            par = nc.snap(par_regs, donate=True, min_val=0, max_val=1)

            # den accumulates (ones/128).T @ et so rbs = recip gives 128/den,
            # folding the fp8 weight scale (x128) into the reciprocal for free
            ones_sq = pp.tile([128, 128], b16, tag="ones_sq", bufs=1)
            nc.vector.memset(ones_sq[:], 1.0 / 128)
            ones8 = pp.tile([128, 2, 128], f8, tag="ones8", bufs=1)
            nc.vector.memset(ones8[:], 1.0)
            tri_t = pp.tile([128, 256], b16, tag="tri", bufs=1)
            ident_t = tri_t[:, 0:128]
            lneg_t = tri_t[:, 128:256]
            # warm the PE HAM clock-gate while the first input DMAs land
            wups = psum()
            for i in range(16):
                nc.tensor.matmul(wups[:, 0:128], ones_sq[:], ones_sq[:],
                                 start=(i == 0), stop=(i == 15))

            # ---- input loads (arrival-ordered for phase-A pipelining).
            # Descriptor generation serializes per issuing queue (~0.7us per
            # dma_start), so the early loads fan out across engine queues.
            def chunks(dram, c0, c1, width):
                return dram.ap()[:, c0 * width:c1 * width].rearrange(
                    "p (c n) -> p c n", n=width)

            wzq_a = pr.tile([128, ND, D], b16, tag="wzq", bufs=1)
            qx_a = pr.tile([128, ND, 1024], b16, tag="qx", bufs=1)
            # single sync queue = priority order at HBM; 4-chunk granules
            # beat the ~0.65us/descriptor issue rate that single-a granules
            # paid, so phase A never catches up with the arrivals
            # single-a granules: issue rate (~0.65us each, alternating) stays
            # ahead of the 8-matmul-per-a consumption rate (~1.7us)
            qxv = qxT_d.ap().rearrange("p (c n) -> p c n", n=1024)
            for a in range(ND):
                nc.sync.dma_start(wzq_a[:, a:a + 1],
                                  chunks(wzq_d, a, a + 1, D))
                nc.sync.dma_start(qx_a[:, a:a + 1, 0:512],
                                  qxv[:, a:a + 1, 0:512])
            nc.sync.dma_start(qx_a[:, :, 512:1024], qxv[:, :, 512:1024])
            # xT feeds the scores pass; host interleaves it so each 512-col
            # chunk is flat-contiguous (full-bandwidth descriptors). Pass 1
            # runs descending from tc15, so load high chunks first.
            xt_a = pb.tile([128, 4, ND, 512], b16, tag="xt", bufs=1)
            for cc in (3, 2, 1, 0):
                nc.sync.dma_start(
                    xt_a[:, cc],
                    xT_d.ap()[:, cc * 4096:(cc + 1) * 4096]
                    .rearrange("p (a n) -> p a n", n=512))
            # x natural layout [t, d] in fp8 feeds the A@X DoubleRow pass
            xn_a = pb.tile([128, NT, D], f8, tag="xn", bufs=1)
            nc.sync.dma_start(xn_a[:], chunks(xn_d, 0, NT, D))
            nc.sync.dma_start(tri_t[:], tri_d.ap())
            # b_ff1 laid out [128, ND]: bias column fc serves f-chunk fc
            bf1_t = pp.tile([128, ND], f32, tag="bf1", bufs=1)
            nc.sync.dma_start(bf1_t[:], bf1_d.ap().rearrange("c p -> p c"))
            wzq_t = [wzq_a[:, d] for d in range(ND)]
            qx = [qx_a[:, d] for d in range(ND)]

            def xts(d, tcn):
                j = tcn % 4
                return xt_a[:, tcn // 4, d, j * 128:(j + 1) * 128]

            # ---- phase A: uT[d, s] = sum_a wzq[a,d] * qxT[a,s] ----
            # a-outer in two sb-halves (8 PSUM banks each, all m per half):
            # compute starts once wzq[a0]+qx[a0,sb0] land, and the sb0 ut
            # evictions (which gate the first scores blocks) overlap the
            # whole sb1 half.
            ut = [pb.tile([128, 1024], b16, tag=f"ut{m}", bufs=1,
                          name=f"ut{m}") for m in range(ND)]

            def phase_a(sb, ms, ups):
                for a in range(ND):
                    for m in ms:
                        nc.tensor.matmul(
                            ups[m][:],
                            wzq_t[a][:, m * 128:(m + 1) * 128],
                            qx[a][:, sb * 512:(sb + 1) * 512],
                            start=(a == 0), stop=(a == ND - 1))

            def evict_u(sb, ms, ups):
                # alternate vector / scalar so the eviction chain halves
                for m in ms:
                    dst = ut[m][:, sb * 512:(sb + 1) * 512]
                    if m % 2 == 0:
                        nc.vector.tensor_copy(dst, ups[m][:])
                    else:
                        nc.scalar.activation(dst, ups[m][:], AF.Copy)

            def phase_a_all():
                for sb in range(2):
                    ms = range(ND)
                    ups = {m: psum() for m in ms}
                    phase_a(sb, ms, ups)
                    evict_u(sb, ms, ups)

            # phase-C weights prefetch into the same right pool (wzq/qx stay
            # live through the in-branch phase-A tail; fp8 weights fit all)
            wl_a = pr.tile([128, ND, D], b16, tag="wl", bufs=1)
            nc.sync.dma_start(wl_a[:], chunks(wvl_d, 0, ND, D))
            wf1_a = pr.tile([128, ND, D], f8, tag="wf1", bufs=1)
            nc.sync.dma_start(wf1_a[:], chunks(wff1_d, 0, ND, D))
            wf2_a = pr.tile([128, ND, D], f8, tag="wf2", bufs=1)
            nc.sync.dma_start(wf2_a[:], chunks(wff2_d, 0, ND, D))
            wvl_t = [wl_a[:, d] for d in range(ND)]

            attn = [pr.tile([128, 1024], b16, tag=f"at{d}", bufs=1,
                            name=f"at{d}") for d in range(ND)]

            def phase_b(parity):
                DR = mybir.MatmulPerfMode.DoubleRow
                # normalized fp8 weights in DoubleRow pair layout, one
                # 512-col buffer per half; groups write their 256-col slice.
                # Diagonal-region pad columns must be zero for paired reads.
                et8 = {}
                for h in range(2):
                    t8 = pb.tile([128, NT // 2, 2, 512], f8, tag=f"et8_{h}",
                                 bufs=1, name=f"et8_{parity}_{h}")
                    et8[h] = t8
                    k0 = HALF_STARTS[parity][h] // 256
                    nc.gpsimd.memset(t8[:, k0, 0, 128:512], 0)
                    nc.gpsimd.memset(t8[:, k0, 1, 256:512], 0)
                    nc.gpsimd.memset(t8[:, k0 + 1, 0, 384:512], 0)

                # pass 1 per group: scoresT -> exp, with the diag mask folded
                # into the scores PSUM as one extra matmul (ident.T @ Lneg
                # adds -30000 below the diagonal) and den accumulated inline
                # one block behind the scores matmuls.
                et = {}
                rbs = {}

                def pass1(g):
                    slots = _gslots(parity, g)[::-1]   # widest first
                    for tcn in slots:
                        w = _width(parity, g, tcn)
                        diag = _is_diag(parity, g, tcn)
                        scp = psum()
                        for d in range(ND):
                            nc.tensor.matmul(
                                scp[:, 0:w],
                                xts(d, tcn),
                                ut[d][:, g * GW:g * GW + w],
                                start=(d == 0),
                                stop=(d == ND - 1 and not diag))
                        if diag:
                            nc.tensor.matmul(
                                scp[:, w - 128:w], ident_t, lneg_t,
                                start=False, stop=True)
                        e = pb.tile([128, w], b16, tag=f"et{g}_{tcn}",
                                    bufs=1, name=f"et{parity}_{g}_{tcn}")
                        et[(g, tcn)] = e
                        nc.scalar.activation(e[:], scp[:, 0:w], AF.Exp)
                    # den as an end-of-group chain: the mm-ring slot it
                    # takes isn't requested again until deep into the next
                    # group's blocks, by which time the recip has read it
                    den_ps = psum()
                    for i, tcn in enumerate(slots):
                        pw = _width(parity, g, tcn)
                        nc.tensor.matmul(
                            den_ps[:, 0:pw], ones_sq[:], et[(g, tcn)][:],
                            start=(i == 0), stop=(i == len(slots) - 1))
                    r = sp.tile([128, GW], f32, tag="rbs", bufs=2,
                                name=f"rbs{parity}_{g}")
                    nc.vector.reciprocal_approx_fast(r[:], den_ps[:, 0:GW])
                    rbs[g] = r

                def quantize(g):
                    # et8 = et * (128/den) into the half buffer's 256-col
                    # slice, alternating vector/gpsimd
                    off = (g % 2) * 256
                    for i, tcn in enumerate(_gslots(parity, g)):
                        w = _width(parity, g, tcn)
                        eng = nc.vector if i % 2 == 0 else nc.gpsimd
                        eng.tensor_mul(
                            et8[g // 2][:, tcn // 2, tcn % 2, off:off + w],
                            et[(g, tcn)][:], rbs[g][:, 0:w])

                def pass2(h):
                    pairs = _hpairs(parity, h)[::-1]   # widest first

                    def pw(k):
                        return _hwidth(parity, h, 2 * k + 1)

                    den2 = psum()
                    for i, k in enumerate(pairs):
                        nc.tensor.matmul(
                            den2[:, 0:pw(k)], ones8[:],
                            et8[h][:, k, :, 0:pw(k)],
                            start=(i == 0), stop=(i == len(pairs) - 1),
                            perf_mode=DR)
                    r2 = sp.tile([128, 512], f32, tag="rbs2", bufs=2,
                                 name=f"rbs2{parity}_{h}")
                    nc.vector.reciprocal_approx_fast(r2[:], den2[:])
                    for dc in range(ND):
                        axp = psum()
                        for i, k in enumerate(pairs):
                            nc.tensor.matmul(
                                axp[:, 0:pw(k)],
                                xn_a[:, 2 * k:2 * k + 2,
                                     dc * 128:(dc + 1) * 128],
                                et8[h][:, k, :, 0:pw(k)],
                                start=(i == 0), stop=(i == len(pairs) - 1),
                                perf_mode=DR)
                        # PSUM reads are DVE-only (GpSimd can't touch PSUM)
                        nc.vector.tensor_mul(
                            attn[dc][:, h * 512:(h + 1) * 512],
                            axp[:], r2[:])

                # software pipeline: quantize(g) (DVE+Pool) hides under
                # later groups' pass-1 and the other half's A@X PE work;
                # recips are approx_fast
                for g in range(4):
                    pass1(g)
                    quantize(g)
                for h in (0, 1):
                    pass2(h)

            # the entire phase A + B sits inside both branch bodies; the
            # branch is resolved right after warmup dispatch, overlapping
            # the startup DMA wait instead of stalling the PE mid-kernel
            with tc.If(par < 1) as cmp:
                phase_a_all()
                phase_b(0)
            with cmp.Else():
                phase_a_all()
                phase_b(1)

            # ---- free pB (ut/xt/xn/et); left pool for phase-C tiles ----
            esB.close()
            esC = es.enter_context(ExitStack())
            pc = esC.enter_context(tc.tile_pool(name="pC", bufs=1))

            x2f = [pc.tile([128, 1024], f32, tag=f"x2f{d}", bufs=1,
                           name=f"x2f{d}") for d in range(ND)]
            # Both FFN GEMMs run in fp8 DoubleRow. Scale chain: x2f carries
            # 32x (host scaled wvl/xq by 32); x2b = x2f/32 is true x2 in fp8;
            # w_ff1/w_ff2 are host-scaled by 32 into fp8's normal range; the
            # relu's scale=1/32 keeps ht exact; the final 32x output factor
            # is divided out on the host.
            x2b_a = pc.tile([128, ND, 1024], f8, tag="x2b", bufs=1)
            ht_a = pc.tile([128, ND, 1024], f8, tag="ht", bufs=1)

            # s2-halved pipeline: ff2(0)'s output adds (DVE) overlap ff1(1)'s
            # matmuls, so only ff2(1)'s tail is exposed past the last matmul
            def wvl_half(s2):
                cc = slice(s2 * 512, (s2 + 1) * 512)
                for oc in range(ND):
                    cps = psum()
                    for d in range(ND):
                        nc.tensor.matmul(
                            cps[:],
                            wvl_t[d][:, oc * 128:(oc + 1) * 128],
                            attn[d][:, cc],
                            start=(d == 0), stop=(d == ND - 1))
                    xqt = sp.tile([128, 512], f32, tag="xqt", bufs=3,
                                  name=f"xqt{oc}_{s2}")
                    nc.sync.dma_start(
                        xqt[:],
                        xq_d.ap()[oc * 128:(oc + 1) * 128, cc])
                    nc.vector.tensor_add(x2f[oc][:, cc], cps[:], xqt[:])
                    # scalar engine does the fp8 extraction; DVE stays free
                    # for the adds and any phase-B eviction spill-over
                    nc.scalar.activation(x2b_a[:, oc, cc], x2f[oc][:, cc],
                                         AF.Copy, scale=1.0 / 32)

            def ff1_half(s2):
                cc = slice(s2 * 512, (s2 + 1) * 512)
                for fc in range(ND):
                    cps = psum()
                    for d2 in range(0, ND, 2):
                        nc.tensor.matmul(
                            cps[:],
                            wf1_a[:, d2:d2 + 2, fc * 128:(fc + 1) * 128],
                            x2b_a[:, d2:d2 + 2, cc],
                            start=(d2 == 0), stop=(d2 == ND - 2),
                            perf_mode=mybir.MatmulPerfMode.DoubleRow)
                    nc.scalar.activation(ht_a[:, fc, cc], cps[:], AF.Relu,
                                         bias=bf1_t[:, fc:fc + 1],
                                         scale=1.0 / 32)

            def ff2_half(s2):
                cc = slice(s2 * 512, (s2 + 1) * 512)
                for oc in range(ND):
                    cps = psum()
                    for f2 in range(0, ND, 2):
                        nc.tensor.matmul(
                            cps[:],
                            wf2_a[:, f2:f2 + 2, oc * 128:(oc + 1) * 128],
                            ht_a[:, f2:f2 + 2, cc],
                            start=(f2 == 0), stop=(f2 == ND - 2),
                            perf_mode=mybir.MatmulPerfMode.DoubleRow)
                    # bf16 staging tile (bufs=8: no reuse wait within a
                    # half) and per-oc DMA on alternating queues: no
                    # serialized descriptor chain, half the exit bytes
                    # bufs=12: half 1's first adds get fresh slots instead
                    # of waiting on half 0's just-issued output transfers
                    ot = sp.tile([128, 512], b16, tag="ot", bufs=12,
                                 name=f"ot{oc}_{s2}")
                    nc.vector.tensor_add(ot[:], cps[:], x2f[oc][:, cc])
                    if s2 == 0:
                        eng = nc.sync if oc % 2 == 0 else nc.gpsimd
                    else:
                        # scalar is free in the last half: a 3rd queue
                        # drains the exit transfers sooner
                        eng = (nc.sync, nc.gpsimd, nc.scalar)[oc % 3]
                    eng.dma_start(
                        outT_d.ap()[oc * 128:(oc + 1) * 128, cc], ot[:])

            wvl_half(0)
            wvl_half(1)
            ff1_half(0)
            ff2_half(0)
            ff1_half(1)
            ff2_half(1)

    nc.compile()
    return nc


def _get_program():
    global _COMPILED
    if _COMPILED is None:
        _COMPILED = _build_program()
    return _COMPILED


def _p128(arr):
    """[c*128, C] -> [128, c*C]: the on-chip chunked layout, so device DMAs
    are contiguous per partition."""
    c = arr.shape[0] // 128
    return np.ascontiguousarray(
        arr.reshape(c, 128, -1).transpose(1, 0, 2).reshape(128, -1))


def _p128_xt(xT):
    """xT [D, S] -> [128, (cc, a, 512)]: 512-col chunks flat-contiguous so
    each chunk loads with full-bandwidth descriptors."""
    a = xT.reshape(ND, 128, 4, 512)            # [a, p, cc, n]
    return np.ascontiguousarray(
        a.transpose(1, 2, 0, 3).reshape(128, -1))


def kernel(x, wqkv, w_lin, b_lin, w_ff1, b_ff1, w_ff2, b_ff2):
    from concourse.bass_utils import run_bass_kernel_spmd

    x = np.asarray(x, np.float32)
    wqkv = np.asarray(wqkv, np.float32)
    Wq = wqkv[:, :D].astype(np.float64)
    Wk = wqkv[:, D:2 * D].astype(np.float64)
    Wv = wqkv[:, 2 * D:].astype(np.float64)

    F8 = ml_dtypes.float8_e4m3
    wzq = _p128(((Wq @ Wk.T) / 2.0).astype(BF16))   # [a, d] natural layout
    # phase C carries a 32x scale (wvl, xq) so x2b = x2f/32 is exact x2;
    # both FFN weights are scaled by 32 into fp8's normal range; the final
    # 32x on the output is divided out below
    wvl = _p128((Wv @ np.asarray(w_lin, np.float64) * 32.0).astype(BF16))
    wff1 = _p128((np.asarray(w_ff1, np.float32) * 32.0).astype(F8))
    wff2 = _p128((np.asarray(w_ff2, np.float32) * 32.0).astype(F8))
    # ident | Lneg: the anti-causal diag mask as a PE-side score bias
    lneg = np.where(np.arange(128)[:, None] < np.arange(128)[None, :],
                    np.float32(-30000.0), np.float32(0.0))
    tri = np.concatenate([np.eye(128, dtype=np.float32), lneg],
                         axis=1).astype(BF16)

    in_maps = []
    qcols_by_parity = {
        0: np.r_[0:512, 1536:2048],
        1: np.r_[512:1536],
    }
    b_lin = np.asarray(b_lin, np.float32)
    b_ff1 = np.asarray(b_ff1, np.float32)
    b_ff2 = np.asarray(b_ff2, np.float32)
    bf1 = np.ascontiguousarray(b_ff1.reshape(ND, 128))
    for c in range(NCORES):
        b, h = c // 2, c % 2
        xT32 = np.ascontiguousarray(x[b].T)               # [D, S] f32
        qcols = qcols_by_parity[h]
        qxT32 = np.ascontiguousarray(xT32[:, qcols])      # [D, 1024]
        in_maps.append({
            "xT": _p128_xt(xT32.astype(BF16)),
            "xn": _p128(x[b].astype(F8)),                 # [S, D] natural
            "qxT": _p128(qxT32.astype(BF16)),
            "xq": (qxT32 + b_lin[:, None]) * 32.0,        # b_lin folded in
            "wzq": wzq,
            "wvl": wvl,
            "wff1": wff1,
            "wff2": wff2,
            "tri": tri,
            "bf1": bf1,
            "par": np.full((1, 1), h, np.uint32),
        })

    global _LAST_IN_MAPS
    _LAST_IN_MAPS = in_maps
    nc = _get_program()
    res = run_bass_kernel_spmd(nc, in_maps, core_ids=list(range(NCORES)))

    out = np.empty((B, S, D), np.float32)
    for c in range(NCORES):
        b, h = c // 2, c % 2
        ol = np.asarray(res.results[c]["outT"],
                        np.float32).T / 32.0              # [1024 s, D]
        if h == 0:
            out[b, 0:512] = ol[:512]
            out[b, 1536:2048] = ol[512:]
        else:
            out[b, 512:1536] = ol
    out += b_ff2[None, None, :]
    return out

